# revision 12
# baseline (speedup 1.0000x reference)
# Trainium2 Bass kernel for AvaAttention (GQA attention + RoPE + additive mask)
# B=2, T=2048, HID=2048, NH=16, KVH=4, HD=128, fp32 — 8 NeuronCores.
#
# Sharding: sequence-parallel. Core i (batch b=i//4, position p=i%4) owns
# q-blocks j = 4s+3-p of batch b, for slot s in 0..3. Projections are
# row-parallel (weights replicated, bf16), K/V exchanged with a SINGLE
# combined AllGather (bf16 payloads packed in a flat fp32 buffer) over
# each batch's 4 cores; attention + output projection stay local.
#
# v3 notes:
#  - Projections run in bf16 (bf16 xT + bf16 weights); RoPE in fp32 from
#    the fp32 PSUM projection result; q/k re-cast to bf16 on the
#    PSUM->SBUF eviction after their transposes.
#  - Scores are computed pre-transposed ([tk, (h tq)]): K block is the
#    stationary operand, 4 q-heads stream at once (N=512). No per-head
#    diag/transpose matmuls, no PSUM->bf16 CAST of probabilities.
#  - Softmax denominators via an all-ones stationary matmul accumulated
#    over kb; result is replicated across partitions so normalization is
#    one elementwise multiply fused into the ctx PSUM->SBUF eviction.
#    1/x via the fast custom-DVE reciprocal (plain reciprocal is ~3.4us
#    per tile and serialized the attention tail).
#  - Additive mask: one N=512 matmul per masked tile (lhsT = mask data in
#    natural [tq, tk], rhs = 4 identity blocks).
#  - Wo is bf16, fully resident in SBUF, and its matmuls interleave with
#    attention per-slot so the tensor engine stays busy to the end.
#  - exp without max-subtraction (safe at this score scale; masked
#    positions hit exp(S-1e9)=0).

import sys

for _p in ("/opt/trn_rl_repo", "/opt/pypackages"):
    if _p not in sys.path:
        sys.path.insert(0, _p)

import numpy as np
import ml_dtypes

B, T, HID = 2, 2048, 2048
NH, KVH, HD = 16, 4, 128
P = 128
NC = 8
NBLK = T // P          # 16 q-blocks per batch
NSLOT = 4              # blocks per core
GPQ = NH // KVH        # 4 q-heads per kv group
HB = HID // P          # 16 contraction subtiles
NEG_THRESH = -1.0e8
KVW = KVH * HD         # 512
SLOT_K = P * KVH * P // 2        # 32768 fp32 words: one slot's bf16 kT
SLOT_V = P * KVW // 2            # 32768 fp32 words: one slot's bf16 V
AG_K = NSLOT * SLOT_K            # 131072
AG_V = NSLOT * SLOT_V            # 131072


def _mask_plan(attention_mask):
    """Classify the additive mask per (j, kb) 128x128 tile.

    Returns (E, P_list): E[s] is the uniform k-extent (in blocks) for slot
    s; P_list is the ordered list of (s, kb) positions where a mask-add is
    applied (positions shared by every core; tile *data* is per-core).
    """
    m = np.asarray(attention_mask).reshape(T, T)
    nonzero = np.zeros((NBLK, NBLK), dtype=bool)
    live = np.zeros((NBLK, NBLK), dtype=bool)   # not fully masked
    for j in range(NBLK):
        for kb in range(NBLK):
            tile = m[j * P:(j + 1) * P, kb * P:(kb + 1) * P]
            nonzero[j, kb] = bool(np.any(tile != 0.0))
            live[j, kb] = bool(np.any(tile > NEG_THRESH))
    kmax = np.ones(NBLK, dtype=int)
    for j in range(NBLK):
        idx = np.nonzero(live[j])[0]
        if len(idx):
            kmax[j] = int(idx[-1]) + 1
    E = [int(max(kmax[4 * s + jj] for jj in range(4))) for s in range(NSLOT)]
    P_list = []
    for s in range(NSLOT):
        for kb in range(E[s]):
            if any(nonzero[4 * s + jj, kb] for jj in range(4)):
                P_list.append((s, kb))
    return E, P_list


def _build_program(E, P_list):
    import concourse.mybir as mybir
    import concourse.tile as tile
    from concourse import bacc
    from concourse.masks import make_identity
    from contextlib import ExitStack

    FP32 = mybir.dt.float32
    FP32R = mybir.dt.float32r
    BF16 = mybir.dt.bfloat16
    Exp = mybir.ActivationFunctionType.Exp
    HALF = HD // 2

    nc = bacc.Bacc("TRN2", target_bir_lowering=False, num_devices=NC)

    x_p = nc.declare_dram_parameter("x", [P, HB * NSLOT * P], BF16, isOutput=False)
    wq_p = nc.declare_dram_parameter("wq", [HID, NH * HD], BF16, isOutput=False)
    wk_p = nc.declare_dram_parameter("wk", [HID, KVH * HD], BF16, isOutput=False)
    wv_p = nc.declare_dram_parameter("wv", [HID, KVH * HD], BF16, isOutput=False)
    wo_p = nc.declare_dram_parameter("wo", [HID, HID], BF16, isOutput=False)
    cosq_p = nc.declare_dram_parameter("cosq", [NSLOT * P, HD], FP32, isOutput=False)
    sinq_p = nc.declare_dram_parameter("sinq3", [NSLOT * P, HD], FP32, isOutput=False)
    cosk_p = nc.declare_dram_parameter("cosk", [NSLOT * P, HD], FP32, isOutput=False)
    sink_p = nc.declare_dram_parameter("sink3", [NSLOT * P, HD], FP32, isOutput=False)
    nmask = max(1, len(P_list))
    masks_p = nc.declare_dram_parameter("masks", [nmask, P, P], BF16, isOutput=False)
    out_p = nc.declare_dram_parameter("out", [NSLOT * P, HID], FP32, isOutput=True)

    HALF_AG = 2 * (SLOT_K + SLOT_V)
    ag_in1 = nc.dram_tensor("ag_in1", [HALF_AG], FP32)
    ag_out1 = nc.dram_tensor("ag_out1", [4, HALF_AG], FP32, addr_space="Local")
    ag_in2 = nc.dram_tensor("ag_in2", [HALF_AG], FP32)
    ag_out2 = nc.dram_tensor("ag_out2", [4, HALF_AG], FP32, addr_space="Local")
    groups = [[0, 1, 2, 3], [4, 5, 6, 7]]

    mask_idx = {sk: idx for idx, sk in enumerate(P_list)}

    def rope(engine, dst, src_ps, cos_t, sin_t, s, nh):
        """dst[t, h, d] = src*cos + rotate_half(src)*sin, natural layout."""
        src3 = src_ps[:].rearrange("p (h d) -> p h d", d=HD)
        cst = rope.pool.tile([P, nh, HD], FP32, name="rope_c", tag="rope_c")
        engine.tensor_tensor(dst[:], src3,
                             cos_t[:, s, None, :].to_broadcast((P, nh, HD)),
                             mybir.AluOpType.mult)
        engine.tensor_tensor(cst[:], src3,
                             sin_t[:, s, None, :].to_broadcast((P, nh, HD)),
                             mybir.AluOpType.mult)
        engine.tensor_tensor(dst[:, :, HALF:], dst[:, :, HALF:],
                             cst[:, :, :HALF], mybir.AluOpType.add)
        engine.tensor_tensor(dst[:, :, :HALF], dst[:, :, :HALF],
                             cst[:, :, HALF:], mybir.AluOpType.add)

    with tile.TileContext(nc) as tc, ExitStack() as top:
        const = top.enter_context(tc.tile_pool(name="const", bufs=1))
        ident_f32 = const.tile([P, P], FP32)
        make_identity(nc, ident_f32[:])
        ones_bf = const.tile([P, P], BF16)
        nc.gpsimd.memset(ones_bf[:], 1.0)

        cosq_t = const.tile([P, NSLOT, HD], FP32)
        sinq_t = const.tile([P, NSLOT, HD], FP32)
        cosk_t = const.tile([P, NSLOT, HD], FP32)
        sink_t = const.tile([P, NSLOT, HD], FP32)
        masks_t = const.tile([P, nmask, P], BF16)

        qT_pool = top.enter_context(tc.tile_pool(name="qT_pool", bufs=1))
        qT = qT_pool.tile([P, NH, NSLOT * P], BF16)           # [d, h, t]

        # ================= projection phases =================
        with tc.tile_pool(name="xT_pool", bufs=1) as xT_pool, \
             tc.tile_pool(name="qw", bufs=1) as qw_pool, \
             tc.tile_pool(name="ph0ps", bufs=2, space="PSUM") as ps0:
            xT = xT_pool.tile([P, HB, NSLOT * P], BF16)       # [h%128, hb, t]
            wq_sb = qw_pool.tile([P, HB, NH * HD], BF16, name="wq_sb")
            # x arrives pre-transposed (host-side) as [p, hb, t] bf16
            for c in range(HB):
                W1 = NSLOT * P
                nc.sync.dma_start(
                    xT[:, c, :], x_p[:, c * W1:(c + 1) * W1])

            # ---- phase 1a: K/V proj + RoPE + per-slot staging + AllGather ----
            with tc.tile_pool(name="kvw", bufs=1) as kvw_pool, \
                 tc.tile_pool(name="kvstage", bufs=2) as kvstage, \
                 tc.tile_pool(name="ktps", bufs=2, space="PSUM") as ktps:
                rope.pool = kvstage
                wk_sb = kvw_pool.tile([P, HB, KVW], BF16, name="wk_sb")
                wv_sb = kvw_pool.tile([P, HB, KVW], BF16, name="wv_sb")
                for c in range(4):
                    C4 = KVW // 4
                    nc.sync.dma_start(
                        wk_sb[:, :, c * C4:(c + 1) * C4],
                        wk_p[:, c * C4:(c + 1) * C4]
                        .rearrange("(hb p) n -> p hb n", p=P))
                    nc.sync.dma_start(
                        wv_sb[:, :, c * C4:(c + 1) * C4],
                        wv_p[:, c * C4:(c + 1) * C4]
                        .rearrange("(hb p) n -> p hb n", p=P))
                for ap, prm in ((cosk_t, cosk_p), (sink_t, sink_p),
                                (cosq_t, cosq_p), (sinq_t, sinq_p)):
                    nc.sync.dma_start(ap[:], prm[:].rearrange("(s p) d -> p s d", p=P))
                nc.sync.dma_start(masks_t[:], masks_p[:].rearrange("n p d -> p n d"))
                for c in range(4):
                    W4 = NH * HD // 4
                    nc.sync.dma_start(
                        wq_sb[:, :, c * W4:(c + 1) * W4],
                        wq_p[:, c * W4:(c + 1) * W4]
                        .rearrange("(hb p) n -> p hb n", p=P))

                for s in range(NSLOT):
                    pk = ps0.tile([P, KVW], FP32, name="pk", tag="pkv")
                    for hb in range(HB):
                        nc.tensor.matmul(pk[:], xT[:, hb, s * P:(s + 1) * P],
                                         wk_sb[:, hb, :],
                                         start=(hb == 0), stop=(hb == HB - 1))
                    kr = kvstage.tile([P, KVH, HD], FP32, name=f"k_rope{s}",
                                      tag=f"k_rope{s % 2}")
                    rope(nc.vector, kr, pk, cosk_t, sink_t, s, KVH)

                    pv = ps0.tile([P, KVW], FP32, name="pv", tag="pkv")
                    for hb in range(HB):
                        nc.tensor.matmul(pv[:], xT[:, hb, s * P:(s + 1) * P],
                                         wv_sb[:, hb, :],
                                         start=(hb == 0), stop=(hb == HB - 1))
                    vst = kvstage.tile([P, KVW], BF16, name=f"v_st{s}", tag="v_st")
                    nc.vector.tensor_copy(vst[:], pv[:])
                    agi, si = (ag_in1, s) if s < 2 else (ag_in2, s - 2)
                    nc.sync.dma_start(
                        agi[2 * SLOT_K + si * SLOT_V:
                            2 * SLOT_K + (si + 1) * SLOT_V]
                        .rearrange("(p w) -> p w", p=P),
                        vst[:].bitcast(FP32))

                    # transpose this slot's k and stage it (bf16)
                    pkt = ktps.tile([P, KVH * P], FP32, name="pkt", tag="pkt")
                    for g in range(KVH):
                        nc.tensor.transpose(pkt[:, g * P:(g + 1) * P],
                                            kr[:, g, :], ident_f32[:])
                    kst = kvstage.tile([P, KVH, P], BF16, name=f"k_st{s}",
                                       tag="k_st")
                    nc.vector.tensor_copy(
                        kst[:], pkt[:].rearrange("p (g t) -> p g t", t=P))
                    nc.sync.dma_start(
                        agi[si * SLOT_K:(si + 1) * SLOT_K]
                        .rearrange("(d g w) -> d g w", d=P, g=KVH),
                        kst[:].bitcast(FP32))
                    if s == 1:
                        nc.gpsimd.collective_compute(
                            "AllGather", mybir.AluOpType.bypass,
                            replica_groups=groups,
                            ins=[ag_in1[:]], outs=[ag_out1[:]])
                    elif s == 3:
                        nc.gpsimd.collective_compute(
                            "AllGather", mybir.AluOpType.bypass,
                            replica_groups=groups,
                            ins=[ag_in2[:]], outs=[ag_out2[:]])

            # ---- phase 1b: Q projection + RoPE + transpose to qT ----
            QC = 4  # heads per Wq chunk
            with tc.tile_pool(name="qstage", bufs=3) as qstage, \
                 tc.tile_pool(name="qps", bufs=2, space="PSUM") as qps, \
                 tc.tile_pool(name="qtps", bufs=2, space="PSUM") as qtps:
                rope.pool = qstage
                for hc in range(NH // QC):
                    q_rope = []
                    for s in range(NSLOT):
                        pq = qps.tile([P, QC * HD], FP32, name="pq", tag="pq")
                        for hb in range(HB):
                            nc.tensor.matmul(pq[:], xT[:, hb, s * P:(s + 1) * P],
                                             wq_sb[:, hb,
                                                   hc * QC * HD:(hc + 1) * QC * HD],
                                             start=(hb == 0), stop=(hb == HB - 1))
                        qr = qstage.tile([P, QC, HD], FP32, name=f"q_rope{s}",
                                         tag=f"q_rope{s % 2}")
                        rope(nc.vector, qr, pq, cosq_t, sinq_t, s, QC)
                        q_rope.append(qr)
                    for h in range(QC):
                        pqt = qtps.tile([P, NSLOT * P], FP32, name="pqt", tag="pqt")
                        for s in range(NSLOT):
                            nc.tensor.transpose(pqt[:, s * P:(s + 1) * P],
                                                q_rope[s][:, h, :], ident_f32[:])
                        nc.vector.tensor_copy(qT[:, hc * QC + h, :], pqt[:])

        # ================= gather + attention + interleaved Wo =================
        with tc.tile_pool(name="kv_pool", bufs=1) as kv_pool, \
             tc.tile_pool(name="wopool", bufs=1) as wopool:
            kT = kv_pool.tile([P, KVH, T], BF16)          # [d, g, t(batch)]
            v_all = kv_pool.tile([P, NBLK, KVW], BF16)    # [t%128, blk, (g d)]

            # block j was produced by in-group position pos=3-(j%4), slot s=j//4
            for j in range(NBLK):
                s, pos = j // 4, 3 - (j % 4)
                ago, si = (ag_out1, s) if s < 2 else (ag_out2, s - 2)
                nc.sync.dma_start(
                    v_all[:, j, :],
                    ago[pos, 2 * SLOT_K + si * SLOT_V:
                        2 * SLOT_K + (si + 1) * SLOT_V]
                    .rearrange("(p w) -> p w", p=P).bitcast(BF16))
                nc.sync.dma_start(
                    kT[:, :, j * P:(j + 1) * P],
                    ago[pos, si * SLOT_K:(si + 1) * SLOT_K]
                    .rearrange("(d g w) -> d g w", d=P, g=KVH).bitcast(BF16))

            wo_sb = wopool.tile([P, HB, HID], BF16, name="wo_sb")
            for c in range(16):
                W16 = HID // 16
                nc.sync.dma_start(
                    wo_sb[:, :, c * W16:(c + 1) * W16],
                    wo_p[:, c * W16:(c + 1) * W16]
                    .rearrange("(hb p) n -> p hb n", p=P))

            with tc.tile_pool(name="ppool", bufs=3) as ppool, \
                 tc.tile_pool(name="astage", bufs=2) as astage, \
                 tc.tile_pool(name="ctxp", bufs=1) as ctxp, \
                 tc.tile_pool(name="ostage", bufs=3) as ostage, \
                 tc.tile_pool(name="scps", bufs=2, space="PSUM") as scps, \
                 tc.tile_pool(name="cps", bufs=2, space="PSUM") as cps, \
                 tc.tile_pool(name="rps", bufs=2, space="PSUM") as rps, \
                 tc.tile_pool(name="ops", bufs=2, space="PSUM") as ops:
                OC = HID // 4

                def wo_chunk(ws, oc, wctx):
                    po = ops.tile([P, OC], FP32, name="po", tag="po")
                    for wg in range(KVH):
                        for wh in range(GPQ):
                            hh = wg * GPQ + wh
                            nc.tensor.matmul(po[:], wctx[:, wg, wh, :],
                                             wo_sb[:, hh, oc * OC:(oc + 1) * OC],
                                             start=(hh == 0), stop=(hh == HB - 1))
                    ot = ostage.tile([P, OC], FP32, name="ot", tag="ot")
                    nc.vector.tensor_copy(ot[:], po[:])
                    nc.sync.dma_start(
                        out_p[ws * P:(ws + 1) * P, oc * OC:(oc + 1) * OC], ot[:])

                for s in range(NSLOT):
                    Es = E[s]
                    ctx_s = ctxp.tile([P, KVH, GPQ, P], BF16, name=f"ctx{s}",
                                      tag=f"ctx{s % 2}")
                    for g in range(KVH):
                        q_rhs = qT[:, g * GPQ:(g + 1) * GPQ, s * P:(s + 1) * P]
                        pctx = cps.tile([P, GPQ * P], FP32, name="pctx", tag="pctx")
                        prs = rps.tile([P, GPQ * P], FP32, name="prs", tag="prs")
                        pt_prev = None
                        for kb in range(Es):
                            psc = scps.tile([P, GPQ * P], FP32, name="psc", tag="psc")
                            mi = mask_idx.get((s, kb))
                            nc.tensor.matmul(
                                psc[:], kT[:, g, kb * P:(kb + 1) * P],
                                q_rhs, start=True, stop=True)
                            if mi is not None:
                                psc3 = psc[:].rearrange("p (h t) -> p h t", t=P)
                                nc.vector.tensor_tensor(
                                    psc3, psc3,
                                    masks_t[:, mi, None, :]
                                    .to_broadcast((P, GPQ, P)),
                                    mybir.AluOpType.add)
                            pt = ppool.tile([P, GPQ * P], BF16, name="pt", tag="pt")
                            nc.scalar.activation(pt[:], psc[:], Exp)
                            nc.tensor.matmul(pctx[:],
                                             v_all[:, kb, g * HD:(g + 1) * HD],
                                             pt[:],
                                             start=(kb == 0), stop=(kb == Es - 1))
                            if kb % 2 == 0:
                                pt_prev = pt
                            else:
                                pp = ppool.tile([P, GPQ * P], BF16,
                                                name="pp", tag="pp")
                                nc.vector.tensor_tensor(pp[:], pt_prev[:], pt[:],
                                                        mybir.AluOpType.add)
                                nc.tensor.matmul(prs[:], ones_bf[:], pp[:],
                                                 start=(kb == 1),
                                                 stop=(kb == Es - 1))
                        rr = astage.tile([P, GPQ * P], FP32, name="rr", tag="rr")
                        nc.vector.reciprocal_approx_fast(rr[:], prs[:])
                        nc.vector.tensor_tensor(
                            ctx_s[:, g, :, :],
                            pctx[:].rearrange("p (h t) -> p h t", t=P),
                            rr[:].rearrange("p (h t) -> p h t", t=P),
                            mybir.AluOpType.mult)
                        # fill exp-wait gaps with the previous slot's Wo chunk
                        if s > 0:
                            wo_chunk(s - 1, g, ctx_prev)
                    ctx_prev = ctx_s
                for oc in range(4):
                    wo_chunk(NSLOT - 1, oc, ctx_prev)

    nc.compile()
    return nc


def _prep_inputs(hidden_states, attention_mask, cos, sin, Wq, Wk, Wv, Wo, P_list):
    hs = np.ascontiguousarray(np.asarray(hidden_states, dtype=np.float32))
    mask = np.asarray(attention_mask, dtype=np.float32).reshape(T, T)
    cos2 = np.asarray(cos, dtype=np.float32).reshape(T, HD)
    sin2 = np.asarray(sin, dtype=np.float32).reshape(T, HD)
    scale = np.float32(1.0 / np.sqrt(HD))

    def t3(s_):
        # rotate_half add trick: t3 = concat(sin[:, 64:], -sin[:, :64])
        return np.concatenate([s_[:, HD // 2:], -s_[:, :HD // 2]], axis=1)

    bf = ml_dtypes.bfloat16
    wq = np.ascontiguousarray(np.asarray(Wq, dtype=np.float32).astype(bf))
    wk = np.ascontiguousarray(np.asarray(Wk, dtype=np.float32).astype(bf))
    wv = np.ascontiguousarray(np.asarray(Wv, dtype=np.float32).astype(bf))
    wo = np.ascontiguousarray(np.asarray(Wo, dtype=np.float32).astype(bf))

    in_maps = []
    for i in range(NC):
        b, pos = i // 4, i % 4
        js = [4 * s + 3 - pos for s in range(NSLOT)]
        take = lambda a: np.ascontiguousarray(
            np.concatenate([a[j * P:(j + 1) * P] for j in js], axis=0))
        m_tiles = [mask[js[s] * P:(js[s] + 1) * P, kb * P:(kb + 1) * P].T
                   for (s, kb) in P_list]
        if not m_tiles:
            m_tiles.append(np.zeros((P, P), np.float32))
        xc = take(hs[b])                         # [512, 2048] fp32
        xt = np.ascontiguousarray(
            xc.T.reshape(HB, P, NSLOT * P).transpose(1, 0, 2)
            .reshape(P, HB * NSLOT * P).astype(bf))
        in_maps.append({
            "x": xt,
            "wq": wq, "wk": wk, "wv": wv, "wo": wo,
            "cosq": take(cos2 * scale),
            "sinq3": take(t3(sin2 * scale)),
            "cosk": take(cos2),
            "sink3": take(t3(sin2)),
            "masks": np.stack(m_tiles).astype(bf),
        })
    return in_maps


_cache = {}


def kernel(hidden_states, attention_mask, cos, sin, Wq, Wk, Wv, Wo,
           _trace=False, _trace_kwargs=None):
    from concourse.bass_utils import run_bass_kernel_spmd

    E, P_list = _mask_plan(attention_mask)
    key = (tuple(E), tuple(P_list))
    if key not in _cache:
        _cache[key] = _build_program(E, P_list)
    nc = _cache[key]

    in_maps = _prep_inputs(hidden_states, attention_mask, cos, sin,
                           Wq, Wk, Wv, Wo, P_list)
    kwargs = dict(_trace_kwargs or {})
    if _trace:
        kwargs["trace"] = True
    res = run_bass_kernel_spmd(nc, in_maps, list(range(NC)), **kwargs)

    out = np.empty((B, T, HID), dtype=np.float32)
    for i in range(NC):
        b, pos = i // 4, i % 4
        o = res.results[i]["out"]
        for s in range(NSLOT):
            j = 4 * s + 3 - pos
            out[b, j * P:(j + 1) * P, :] = o[s * P:(s + 1) * P, :]
    kernel._last_result = res
    return out


# revision 14
# speedup vs baseline: 1.0934x; 1.0934x over previous
# Trainium2 Bass kernel for AvaAttention (GQA attention + RoPE + additive mask)
# B=2, T=2048, HID=2048, NH=16, KVH=4, HD=128, fp32 — 8 NeuronCores.
#
# Sharding: sequence-parallel. Core i (batch b=i//4, position p=i%4) owns
# q-blocks j = 4s+3-p of batch b, for slot s in 0..3. Projections are
# row-parallel (weights replicated, bf16), K/V exchanged with a SINGLE
# combined AllGather (bf16 payloads packed in a flat fp32 buffer) over
# each batch's 4 cores; attention + output projection stay local.
#
# v3 notes:
#  - Projections run in bf16 (bf16 xT + bf16 weights); RoPE in fp32 from
#    the fp32 PSUM projection result; q/k re-cast to bf16 on the
#    PSUM->SBUF eviction after their transposes.
#  - Scores are computed pre-transposed ([tk, (h tq)]): K block is the
#    stationary operand, 4 q-heads stream at once (N=512). No per-head
#    diag/transpose matmuls, no PSUM->bf16 CAST of probabilities.
#  - Softmax denominators via an all-ones stationary matmul accumulated
#    over kb; result is replicated across partitions so normalization is
#    one elementwise multiply fused into the ctx PSUM->SBUF eviction.
#    1/x via the fast custom-DVE reciprocal (plain reciprocal is ~3.4us
#    per tile and serialized the attention tail).
#  - Additive mask: one N=512 matmul per masked tile (lhsT = mask data in
#    natural [tq, tk], rhs = 4 identity blocks).
#  - Wo is bf16, fully resident in SBUF, and its matmuls interleave with
#    attention per-slot so the tensor engine stays busy to the end.
#  - exp without max-subtraction (safe at this score scale; masked
#    positions hit exp(S-1e9)=0).

import sys

for _p in ("/opt/trn_rl_repo", "/opt/pypackages"):
    if _p not in sys.path:
        sys.path.insert(0, _p)

import numpy as np
import ml_dtypes

B, T, HID = 2, 2048, 2048
NH, KVH, HD = 16, 4, 128
P = 128
NC = 8
NBLK = T // P          # 16 q-blocks per batch
NSLOT = 4              # blocks per core
GPQ = NH // KVH        # 4 q-heads per kv group
HB = HID // P          # 16 contraction subtiles
NEG_THRESH = -1.0e8
KVW = KVH * HD         # 512
SLOT_K = P * KVH * P // 2        # 32768 fp32 words: one slot's bf16 kT
SLOT_V = P * KVW // 2            # 32768 fp32 words: one slot's bf16 V
AG_K = NSLOT * SLOT_K            # 131072
AG_V = NSLOT * SLOT_V            # 131072


def _mask_plan(attention_mask):
    """Classify the additive mask per (j, kb) 128x128 tile.

    Returns (E, P_list): E[s] is the uniform k-extent (in blocks) for slot
    s; P_list is the ordered list of (s, kb) positions where a mask-add is
    applied (positions shared by every core; tile *data* is per-core).
    """
    m = np.asarray(attention_mask).reshape(T, T)
    nonzero = np.zeros((NBLK, NBLK), dtype=bool)
    live = np.zeros((NBLK, NBLK), dtype=bool)   # not fully masked
    for j in range(NBLK):
        for kb in range(NBLK):
            tile = m[j * P:(j + 1) * P, kb * P:(kb + 1) * P]
            nonzero[j, kb] = bool(np.any(tile != 0.0))
            live[j, kb] = bool(np.any(tile > NEG_THRESH))
    kmax = np.ones(NBLK, dtype=int)
    for j in range(NBLK):
        idx = np.nonzero(live[j])[0]
        if len(idx):
            kmax[j] = int(idx[-1]) + 1
    E = [int(max(kmax[4 * s + jj] for jj in range(4))) for s in range(NSLOT)]
    P_list = []
    for s in range(NSLOT):
        for kb in range(E[s]):
            if any(nonzero[4 * s + jj, kb] for jj in range(4)):
                P_list.append((s, kb))
    return E, P_list


def _build_program(E, P_list):
    import concourse.mybir as mybir
    import concourse.tile as tile
    from concourse import bacc
    from concourse.masks import make_identity
    from contextlib import ExitStack

    FP32 = mybir.dt.float32
    FP32R = mybir.dt.float32r
    BF16 = mybir.dt.bfloat16
    Exp = mybir.ActivationFunctionType.Exp
    HALF = HD // 2

    nc = bacc.Bacc("TRN2", target_bir_lowering=False, num_devices=NC)

    x_p = nc.declare_dram_parameter("x", [P, HB * NSLOT * P], BF16, isOutput=False)
    wq_p = nc.declare_dram_parameter("wq", [P, HB * NH * HD], BF16, isOutput=False)
    wk_p = nc.declare_dram_parameter("wk", [P, HB * KVH * HD], BF16, isOutput=False)
    wv_p = nc.declare_dram_parameter("wv", [P, HB * KVH * HD], BF16, isOutput=False)
    wo_p = nc.declare_dram_parameter("wo", [P, HB * HID], BF16, isOutput=False)
    cosq_p = nc.declare_dram_parameter("cosq", [NSLOT * P, HD], FP32, isOutput=False)
    sinq_p = nc.declare_dram_parameter("sinq3", [NSLOT * P, HD], FP32, isOutput=False)
    cosk_p = nc.declare_dram_parameter("cosk", [NSLOT * P, HD], FP32, isOutput=False)
    sink_p = nc.declare_dram_parameter("sink3", [NSLOT * P, HD], FP32, isOutput=False)
    nmask = max(1, len(P_list))
    masks_p = nc.declare_dram_parameter("masks", [nmask, P, P], BF16, isOutput=False)
    out_p = nc.declare_dram_parameter("out", [NSLOT * P, HID], FP32, isOutput=True)

    HALF_AG = 2 * (SLOT_K + SLOT_V)
    ag_in1 = nc.dram_tensor("ag_in1", [HALF_AG], FP32)
    ag_out1 = nc.dram_tensor("ag_out1", [4, HALF_AG], FP32, addr_space="Local")
    ag_in2 = nc.dram_tensor("ag_in2", [HALF_AG], FP32)
    ag_out2 = nc.dram_tensor("ag_out2", [4, HALF_AG], FP32, addr_space="Local")
    groups = [[0, 1, 2, 3], [4, 5, 6, 7]]

    mask_idx = {sk: idx for idx, sk in enumerate(P_list)}

    def rope(engine, dst, src_ps, cos_t, sin_t, s, nh):
        """dst[t, h, d] = src*cos + rotate_half(src)*sin, natural layout."""
        src3 = src_ps[:].rearrange("p (h d) -> p h d", d=HD)
        cst = rope.pool.tile([P, nh, HD], FP32, name="rope_c", tag="rope_c")
        engine.tensor_tensor(dst[:], src3,
                             cos_t[:, s, None, :].to_broadcast((P, nh, HD)),
                             mybir.AluOpType.mult)
        engine.tensor_tensor(cst[:], src3,
                             sin_t[:, s, None, :].to_broadcast((P, nh, HD)),
                             mybir.AluOpType.mult)
        engine.tensor_tensor(dst[:, :, HALF:], dst[:, :, HALF:],
                             cst[:, :, :HALF], mybir.AluOpType.add)
        engine.tensor_tensor(dst[:, :, :HALF], dst[:, :, :HALF],
                             cst[:, :, HALF:], mybir.AluOpType.add)

    with tile.TileContext(nc) as tc, ExitStack() as top:
        const = top.enter_context(tc.tile_pool(name="const", bufs=1))
        ident_f32 = const.tile([P, P], FP32)
        make_identity(nc, ident_f32[:])
        ones_bf = const.tile([P, P], BF16)
        nc.gpsimd.memset(ones_bf[:], 1.0)

        cosq_t = const.tile([P, NSLOT, HD], FP32)
        sinq_t = const.tile([P, NSLOT, HD], FP32)
        cosk_t = const.tile([P, NSLOT, HD], FP32)
        sink_t = const.tile([P, NSLOT, HD], FP32)
        masks_t = const.tile([P, nmask, P], BF16)

        qT_pool = top.enter_context(tc.tile_pool(name="qT_pool", bufs=1))
        qT = qT_pool.tile([P, NH, NSLOT * P], BF16)           # [d, h, t]

        # ================= projection phases =================
        with tc.tile_pool(name="xT_pool", bufs=1) as xT_pool, \
             tc.tile_pool(name="qw", bufs=1) as qw_pool, \
             tc.tile_pool(name="ph0ps", bufs=2, space="PSUM") as ps0:
            xT = xT_pool.tile([P, HB, NSLOT * P], BF16)       # [h%128, hb, t]
            wq_sb = qw_pool.tile([P, HB, NH * HD], BF16, name="wq_sb")
            # x arrives pre-transposed (host-side) as [p, hb, t] bf16
            for c in range(2):
                HBH = HB // 2
                WH = HBH * NSLOT * P
                nc.sync.dma_start(
                    xT[:, c * HBH:(c + 1) * HBH, :],
                    x_p[:, c * WH:(c + 1) * WH]
                    .rearrange("p (hb t) -> p hb t", hb=HBH))

            # ---- phase 1a: K/V proj + RoPE + per-slot staging + AllGather ----
            with tc.tile_pool(name="kvw", bufs=1) as kvw_pool, \
                 tc.tile_pool(name="kvstage", bufs=2) as kvstage, \
                 tc.tile_pool(name="ktps", bufs=2, space="PSUM") as ktps:
                rope.pool = kvstage
                wk_sb = kvw_pool.tile([P, HB, KVW], BF16, name="wk_sb")
                wv_sb = kvw_pool.tile([P, HB, KVW], BF16, name="wv_sb")
                for c in range(2):
                    HBH = HB // 2
                    WH = HBH * KVW
                    nc.sync.dma_start(
                        wk_sb[:, c * HBH:(c + 1) * HBH, :],
                        wk_p[:, c * WH:(c + 1) * WH]
                        .rearrange("p (hb n) -> p hb n", hb=HBH))
                    nc.sync.dma_start(
                        wv_sb[:, c * HBH:(c + 1) * HBH, :],
                        wv_p[:, c * WH:(c + 1) * WH]
                        .rearrange("p (hb n) -> p hb n", hb=HBH))
                for ap, prm in ((cosk_t, cosk_p), (sink_t, sink_p),
                                (cosq_t, cosq_p), (sinq_t, sinq_p)):
                    nc.sync.dma_start(ap[:], prm[:].rearrange("(s p) d -> p s d", p=P))
                nc.sync.dma_start(masks_t[:], masks_p[:].rearrange("n p d -> p n d"))
                for c in range(4):
                    HBQ = HB // 4
                    WQ4 = HBQ * NH * HD
                    nc.sync.dma_start(
                        wq_sb[:, c * HBQ:(c + 1) * HBQ, :],
                        wq_p[:, c * WQ4:(c + 1) * WQ4]
                        .rearrange("p (hb n) -> p hb n", hb=HBQ))

                for s in range(NSLOT):
                    pk = ps0.tile([P, KVW], FP32, name="pk", tag="pkv")
                    for hb in range(HB):
                        nc.tensor.matmul(pk[:], xT[:, hb, s * P:(s + 1) * P],
                                         wk_sb[:, hb, :],
                                         start=(hb == 0), stop=(hb == HB - 1))
                    kr = kvstage.tile([P, KVH, HD], FP32, name=f"k_rope{s}",
                                      tag=f"k_rope{s % 2}")
                    rope(nc.vector, kr, pk, cosk_t, sink_t, s, KVH)

                    pv = ps0.tile([P, KVW], FP32, name="pv", tag="pkv")
                    for hb in range(HB):
                        nc.tensor.matmul(pv[:], xT[:, hb, s * P:(s + 1) * P],
                                         wv_sb[:, hb, :],
                                         start=(hb == 0), stop=(hb == HB - 1))
                    vst = kvstage.tile([P, KVW], BF16, name=f"v_st{s}", tag="v_st")
                    nc.vector.tensor_copy(vst[:], pv[:])
                    agi, si = (ag_in1, s) if s < 2 else (ag_in2, s - 2)
                    nc.sync.dma_start(
                        agi[2 * SLOT_K + si * SLOT_V:
                            2 * SLOT_K + (si + 1) * SLOT_V]
                        .rearrange("(p w) -> p w", p=P),
                        vst[:].bitcast(FP32))

                    # transpose this slot's k and stage it (bf16)
                    pkt = ktps.tile([P, KVH * P], FP32, name="pkt", tag="pkt")
                    for g in range(KVH):
                        nc.tensor.transpose(pkt[:, g * P:(g + 1) * P],
                                            kr[:, g, :], ident_f32[:])
                    kst = kvstage.tile([P, KVH, P], BF16, name=f"k_st{s}",
                                       tag="k_st")
                    nc.vector.tensor_copy(
                        kst[:], pkt[:].rearrange("p (g t) -> p g t", t=P))
                    nc.sync.dma_start(
                        agi[si * SLOT_K:(si + 1) * SLOT_K]
                        .rearrange("(d g w) -> d g w", d=P, g=KVH),
                        kst[:].bitcast(FP32))
                    if s == 1:
                        nc.gpsimd.collective_compute(
                            "AllGather", mybir.AluOpType.bypass,
                            replica_groups=groups,
                            ins=[ag_in1[:]], outs=[ag_out1[:]])
                    elif s == 3:
                        nc.gpsimd.collective_compute(
                            "AllGather", mybir.AluOpType.bypass,
                            replica_groups=groups,
                            ins=[ag_in2[:]], outs=[ag_out2[:]])

            # ---- phase 1b: Q projection + RoPE + transpose to qT ----
            QC = 4  # heads per Wq chunk
            with tc.tile_pool(name="qstage", bufs=3) as qstage, \
                 tc.tile_pool(name="qps", bufs=2, space="PSUM") as qps, \
                 tc.tile_pool(name="qtps", bufs=2, space="PSUM") as qtps:
                rope.pool = qstage
                for hc in range(NH // QC):
                    q_rope = []
                    for s in range(NSLOT):
                        pq = qps.tile([P, QC * HD], FP32, name="pq", tag="pq")
                        for hb in range(HB):
                            nc.tensor.matmul(pq[:], xT[:, hb, s * P:(s + 1) * P],
                                             wq_sb[:, hb,
                                                   hc * QC * HD:(hc + 1) * QC * HD],
                                             start=(hb == 0), stop=(hb == HB - 1))
                        qr = qstage.tile([P, QC, HD], FP32, name=f"q_rope{s}",
                                         tag=f"q_rope{s % 2}")
                        rope(nc.vector, qr, pq, cosq_t, sinq_t, s, QC)
                        q_rope.append(qr)
                    for h in range(QC):
                        pqt = qtps.tile([P, NSLOT * P], FP32, name="pqt", tag="pqt")
                        for s in range(NSLOT):
                            nc.tensor.transpose(pqt[:, s * P:(s + 1) * P],
                                                q_rope[s][:, h, :], ident_f32[:])
                        nc.vector.tensor_copy(qT[:, hc * QC + h, :], pqt[:])

        # ================= gather + attention + interleaved Wo =================
        with tc.tile_pool(name="kv_pool", bufs=1) as kv_pool, \
             tc.tile_pool(name="wopool", bufs=1) as wopool:
            kT = kv_pool.tile([P, KVH, T], BF16)          # [d, g, t(batch)]
            v_all = kv_pool.tile([P, NBLK, KVW], BF16)    # [t%128, blk, (g d)]

            # block j was produced by in-group position pos=3-(j%4), slot s=j//4
            for j in range(NBLK):
                s, pos = j // 4, 3 - (j % 4)
                ago, si = (ag_out1, s) if s < 2 else (ag_out2, s - 2)
                nc.sync.dma_start(
                    v_all[:, j, :],
                    ago[pos, 2 * SLOT_K + si * SLOT_V:
                        2 * SLOT_K + (si + 1) * SLOT_V]
                    .rearrange("(p w) -> p w", p=P).bitcast(BF16))
                nc.sync.dma_start(
                    kT[:, :, j * P:(j + 1) * P],
                    ago[pos, si * SLOT_K:(si + 1) * SLOT_K]
                    .rearrange("(d g w) -> d g w", d=P, g=KVH).bitcast(BF16))

            wo_sb = wopool.tile([P, HB, HID], BF16, name="wo_sb")
            for c in range(4):
                HBQ = HB // 4
                WO4 = HBQ * HID
                nc.sync.dma_start(
                    wo_sb[:, c * HBQ:(c + 1) * HBQ, :],
                    wo_p[:, c * WO4:(c + 1) * WO4]
                    .rearrange("p (hb n) -> p hb n", hb=HBQ))

            with tc.tile_pool(name="ppool", bufs=3) as ppool, \
                 tc.tile_pool(name="astage", bufs=2) as astage, \
                 tc.tile_pool(name="ctxp", bufs=1) as ctxp, \
                 tc.tile_pool(name="ostage", bufs=3) as ostage, \
                 tc.tile_pool(name="scps", bufs=2, space="PSUM") as scps, \
                 tc.tile_pool(name="cps", bufs=2, space="PSUM") as cps, \
                 tc.tile_pool(name="rps", bufs=2, space="PSUM") as rps, \
                 tc.tile_pool(name="ops", bufs=2, space="PSUM") as ops:
                OC = HID // 4

                def wo_chunk(ws, oc, wctx):
                    po = ops.tile([P, OC], FP32, name="po", tag="po")
                    for wg in range(KVH):
                        for wh in range(GPQ):
                            hh = wg * GPQ + wh
                            nc.tensor.matmul(po[:], wctx[:, wg, wh, :],
                                             wo_sb[:, hh, oc * OC:(oc + 1) * OC],
                                             start=(hh == 0), stop=(hh == HB - 1))
                    ot = ostage.tile([P, OC], FP32, name="ot", tag="ot")
                    nc.vector.tensor_copy(ot[:], po[:])
                    nc.sync.dma_start(
                        out_p[ws * P:(ws + 1) * P, oc * OC:(oc + 1) * OC], ot[:])

                for s in range(NSLOT):
                    Es = E[s]
                    ctx_s = ctxp.tile([P, KVH, GPQ, P], BF16, name=f"ctx{s}",
                                      tag=f"ctx{s % 2}")
                    for g in range(KVH):
                        q_rhs = qT[:, g * GPQ:(g + 1) * GPQ, s * P:(s + 1) * P]
                        pctx = cps.tile([P, GPQ * P], FP32, name="pctx", tag="pctx")
                        prs = rps.tile([P, GPQ * P], FP32, name="prs", tag="prs")
                        pt_prev = None
                        for kb in range(Es):
                            psc = scps.tile([P, GPQ * P], FP32, name="psc", tag="psc")
                            mi = mask_idx.get((s, kb))
                            nc.tensor.matmul(
                                psc[:], kT[:, g, kb * P:(kb + 1) * P],
                                q_rhs, start=True, stop=True)
                            if mi is not None:
                                psc3 = psc[:].rearrange("p (h t) -> p h t", t=P)
                                nc.vector.tensor_tensor(
                                    psc3, psc3,
                                    masks_t[:, mi, None, :]
                                    .to_broadcast((P, GPQ, P)),
                                    mybir.AluOpType.add)
                            pt = ppool.tile([P, GPQ * P], BF16, name="pt", tag="pt")
                            nc.scalar.activation(pt[:], psc[:], Exp)
                            nc.tensor.matmul(pctx[:],
                                             v_all[:, kb, g * HD:(g + 1) * HD],
                                             pt[:],
                                             start=(kb == 0), stop=(kb == Es - 1))
                            if kb % 2 == 0:
                                pt_prev = pt
                            else:
                                pp = ppool.tile([P, GPQ * P], BF16,
                                                name="pp", tag="pp")
                                nc.vector.tensor_tensor(pp[:], pt_prev[:], pt[:],
                                                        mybir.AluOpType.add)
                                nc.tensor.matmul(prs[:], ones_bf[:], pp[:],
                                                 start=(kb == 1),
                                                 stop=(kb == Es - 1))
                        rr = astage.tile([P, GPQ * P], FP32, name="rr", tag="rr")
                        nc.vector.reciprocal_approx_fast(rr[:], prs[:])
                        nc.vector.tensor_tensor(
                            ctx_s[:, g, :, :],
                            pctx[:].rearrange("p (h t) -> p h t", t=P),
                            rr[:].rearrange("p (h t) -> p h t", t=P),
                            mybir.AluOpType.mult)
                        # fill exp-wait gaps with the previous slot's Wo chunk
                        if s > 0:
                            wo_chunk(s - 1, g, ctx_prev)
                    ctx_prev = ctx_s
                for oc in range(4):
                    wo_chunk(NSLOT - 1, oc, ctx_prev)

    nc.compile()
    return nc


def _prep_inputs(hidden_states, attention_mask, cos, sin, Wq, Wk, Wv, Wo, P_list):
    hs = np.ascontiguousarray(np.asarray(hidden_states, dtype=np.float32))
    mask = np.asarray(attention_mask, dtype=np.float32).reshape(T, T)
    cos2 = np.asarray(cos, dtype=np.float32).reshape(T, HD)
    sin2 = np.asarray(sin, dtype=np.float32).reshape(T, HD)
    scale = np.float32(1.0 / np.sqrt(HD))

    def t3(s_):
        # rotate_half add trick: t3 = concat(sin[:, 64:], -sin[:, :64])
        return np.concatenate([s_[:, HD // 2:], -s_[:, :HD // 2]], axis=1)

    bf = ml_dtypes.bfloat16

    def wprep(w):
        w = np.asarray(w, dtype=np.float32).astype(bf)
        n = w.shape[1]
        return np.ascontiguousarray(
            w.reshape(HB, P, n).transpose(1, 0, 2).reshape(P, HB * n))

    wq = wprep(Wq)
    wk = wprep(Wk)
    wv = wprep(Wv)
    wo = wprep(Wo)

    in_maps = []
    for i in range(NC):
        b, pos = i // 4, i % 4
        js = [4 * s + 3 - pos for s in range(NSLOT)]
        take = lambda a: np.ascontiguousarray(
            np.concatenate([a[j * P:(j + 1) * P] for j in js], axis=0))
        m_tiles = [mask[js[s] * P:(js[s] + 1) * P, kb * P:(kb + 1) * P].T
                   for (s, kb) in P_list]
        if not m_tiles:
            m_tiles.append(np.zeros((P, P), np.float32))
        xc = take(hs[b])                         # [512, 2048] fp32
        xt = np.ascontiguousarray(
            xc.T.reshape(HB, P, NSLOT * P).transpose(1, 0, 2)
            .reshape(P, HB * NSLOT * P).astype(bf))
        in_maps.append({
            "x": xt,
            "wq": wq, "wk": wk, "wv": wv, "wo": wo,
            "cosq": take(cos2 * scale),
            "sinq3": take(t3(sin2 * scale)),
            "cosk": take(cos2),
            "sink3": take(t3(sin2)),
            "masks": np.stack(m_tiles).astype(bf),
        })
    return in_maps


_cache = {}


def kernel(hidden_states, attention_mask, cos, sin, Wq, Wk, Wv, Wo,
           _trace=False, _trace_kwargs=None):
    from concourse.bass_utils import run_bass_kernel_spmd

    E, P_list = _mask_plan(attention_mask)
    key = (tuple(E), tuple(P_list))
    if key not in _cache:
        _cache[key] = _build_program(E, P_list)
    nc = _cache[key]

    in_maps = _prep_inputs(hidden_states, attention_mask, cos, sin,
                           Wq, Wk, Wv, Wo, P_list)
    kwargs = dict(_trace_kwargs or {})
    if _trace:
        kwargs["trace"] = True
    res = run_bass_kernel_spmd(nc, in_maps, list(range(NC)), **kwargs)

    out = np.empty((B, T, HID), dtype=np.float32)
    for i in range(NC):
        b, pos = i // 4, i % 4
        o = res.results[i]["out"]
        for s in range(NSLOT):
            j = 4 * s + 3 - pos
            out[b, j * P:(j + 1) * P, :] = o[s * P:(s + 1) * P, :]
    kernel._last_result = res
    return out


# revision 19
# speedup vs baseline: 1.0935x; 1.0001x over previous
# Trainium2 Bass kernel for AvaAttention (GQA attention + RoPE + additive mask)
# B=2, T=2048, HID=2048, NH=16, KVH=4, HD=128, fp32 — 8 NeuronCores.
#
# Sharding: sequence-parallel. Core i (batch b=i//4, position p=i%4) owns
# q-blocks j = 4s+3-p of batch b, for slot s in 0..3. Projections are
# row-parallel (weights replicated, bf16), K/V exchanged with a SINGLE
# combined AllGather (bf16 payloads packed in a flat fp32 buffer) over
# each batch's 4 cores; attention + output projection stay local.
#
# v3 notes:
#  - Projections run in bf16 (bf16 xT + bf16 weights); RoPE in fp32 from
#    the fp32 PSUM projection result; q/k re-cast to bf16 on the
#    PSUM->SBUF eviction after their transposes.
#  - Scores are computed pre-transposed ([tk, (h tq)]): K block is the
#    stationary operand, 4 q-heads stream at once (N=512). No per-head
#    diag/transpose matmuls, no PSUM->bf16 CAST of probabilities.
#  - Softmax denominators via an all-ones stationary matmul accumulated
#    over kb; result is replicated across partitions so normalization is
#    one elementwise multiply fused into the ctx PSUM->SBUF eviction.
#    1/x via the fast custom-DVE reciprocal (plain reciprocal is ~3.4us
#    per tile and serialized the attention tail).
#  - Additive mask: one N=512 matmul per masked tile (lhsT = mask data in
#    natural [tq, tk], rhs = 4 identity blocks).
#  - Wo is bf16, fully resident in SBUF, and its matmuls interleave with
#    attention per-slot so the tensor engine stays busy to the end.
#  - exp without max-subtraction (safe at this score scale; masked
#    positions hit exp(S-1e9)=0).

import sys

for _p in ("/opt/trn_rl_repo", "/opt/pypackages"):
    if _p not in sys.path:
        sys.path.insert(0, _p)

import numpy as np
import ml_dtypes

B, T, HID = 2, 2048, 2048
NH, KVH, HD = 16, 4, 128
P = 128
NC = 8
NBLK = T // P          # 16 q-blocks per batch
NSLOT = 4              # blocks per core
GPQ = NH // KVH        # 4 q-heads per kv group
HB = HID // P          # 16 contraction subtiles
NEG_THRESH = -1.0e8
KVW = KVH * HD         # 512
SLOT_K = P * KVH * P // 2        # 32768 fp32 words: one slot's bf16 kT
SLOT_V = P * KVW // 2            # 32768 fp32 words: one slot's bf16 V
AG_K = NSLOT * SLOT_K            # 131072
AG_V = NSLOT * SLOT_V            # 131072


def _mask_plan(attention_mask):
    """Classify the additive mask per (j, kb) 128x128 tile.

    Returns (E, P_list): E[s] is the uniform k-extent (in blocks) for slot
    s; P_list is the ordered list of (s, kb) positions where a mask-add is
    applied (positions shared by every core; tile *data* is per-core).
    """
    m = np.asarray(attention_mask).reshape(T, T)
    nonzero = np.zeros((NBLK, NBLK), dtype=bool)
    live = np.zeros((NBLK, NBLK), dtype=bool)   # not fully masked
    for j in range(NBLK):
        for kb in range(NBLK):
            tile = m[j * P:(j + 1) * P, kb * P:(kb + 1) * P]
            nonzero[j, kb] = bool(np.any(tile != 0.0))
            live[j, kb] = bool(np.any(tile > NEG_THRESH))
    kmax = np.ones(NBLK, dtype=int)
    for j in range(NBLK):
        idx = np.nonzero(live[j])[0]
        if len(idx):
            kmax[j] = int(idx[-1]) + 1
    E = [int(max(kmax[4 * s + jj] for jj in range(4))) for s in range(NSLOT)]
    P_list = []
    for s in range(NSLOT):
        for kb in range(E[s]):
            if any(nonzero[4 * s + jj, kb] for jj in range(4)):
                P_list.append((s, kb))
    return E, P_list


def _build_program(E, P_list):
    import concourse.mybir as mybir
    import concourse.tile as tile
    from concourse import bacc
    from concourse.masks import make_identity
    from contextlib import ExitStack

    FP32 = mybir.dt.float32
    FP32R = mybir.dt.float32r
    BF16 = mybir.dt.bfloat16
    FP8 = mybir.dt.float8e4
    DR = mybir.MatmulPerfMode.DoubleRow
    Exp = mybir.ActivationFunctionType.Exp
    HALF = HD // 2

    nc = bacc.Bacc("TRN2", target_bir_lowering=False, num_devices=NC)

    x_p = nc.declare_dram_parameter("x", [P, HB * NSLOT * P], BF16, isOutput=False)
    wq_p = nc.declare_dram_parameter("wq", [P, HB * NH * HD], BF16, isOutput=False)
    wk_p = nc.declare_dram_parameter("wk", [P, HB * KVH * HD], BF16, isOutput=False)
    wv_p = nc.declare_dram_parameter("wv", [P, HB * KVH * HD], BF16, isOutput=False)
    wo_p = nc.declare_dram_parameter("wo", [P, HB * HID], BF16, isOutput=False)
    cosq_p = nc.declare_dram_parameter("cosq", [NSLOT * P, HD], FP32, isOutput=False)
    sinq_p = nc.declare_dram_parameter("sinq3", [NSLOT * P, HD], FP32, isOutput=False)
    cosk_p = nc.declare_dram_parameter("cosk", [NSLOT * P, HD], FP32, isOutput=False)
    sink_p = nc.declare_dram_parameter("sink3", [NSLOT * P, HD], FP32, isOutput=False)
    nmask = max(1, len(P_list))
    masks_p = nc.declare_dram_parameter("masks", [nmask, P, P], BF16, isOutput=False)
    out_p = nc.declare_dram_parameter("out", [NSLOT * P, HID], FP32, isOutput=True)

    HALF_AG = 2 * (SLOT_K + SLOT_V)
    ag_in1 = nc.dram_tensor("ag_in1", [HALF_AG], FP32)
    ag_out1 = nc.dram_tensor("ag_out1", [4, HALF_AG], FP32, addr_space="Local")
    ag_in2 = nc.dram_tensor("ag_in2", [HALF_AG], FP32)
    ag_out2 = nc.dram_tensor("ag_out2", [4, HALF_AG], FP32, addr_space="Local")
    groups = [[0, 1, 2, 3], [4, 5, 6, 7]]

    mask_idx = {sk: idx for idx, sk in enumerate(P_list)}

    def rope(engine, dst, src_ps, cos_t, sin_t, s, nh):
        """dst[t, h, d] = src*cos + rotate_half(src)*sin, natural layout."""
        src3 = src_ps[:].rearrange("p (h d) -> p h d", d=HD)
        cst = rope.pool.tile([P, nh, HD], FP32, name="rope_c", tag="rope_c")
        engine.tensor_tensor(dst[:], src3,
                             cos_t[:, s, None, :].to_broadcast((P, nh, HD)),
                             mybir.AluOpType.mult)
        engine.tensor_tensor(cst[:], src3,
                             sin_t[:, s, None, :].to_broadcast((P, nh, HD)),
                             mybir.AluOpType.mult)
        engine.tensor_tensor(dst[:, :, HALF:], dst[:, :, HALF:],
                             cst[:, :, :HALF], mybir.AluOpType.add)
        engine.tensor_tensor(dst[:, :, :HALF], dst[:, :, :HALF],
                             cst[:, :, HALF:], mybir.AluOpType.add)

    with tile.TileContext(nc) as tc, ExitStack() as top:
        const = top.enter_context(tc.tile_pool(name="const", bufs=1))
        ident_f32 = const.tile([P, P], FP32)
        make_identity(nc, ident_f32[:])
        ones_bf = const.tile([P, P], BF16)
        nc.gpsimd.memset(ones_bf[:], 1.0)

        cosq_t = const.tile([P, NSLOT, HD], FP32)
        sinq_t = const.tile([P, NSLOT, HD], FP32)
        cosk_t = const.tile([P, NSLOT, HD], FP32)
        sink_t = const.tile([P, NSLOT, HD], FP32)
        masks_t = const.tile([P, nmask, P], BF16)

        qT_pool = top.enter_context(tc.tile_pool(name="qT_pool", bufs=1))
        qT = qT_pool.tile([P, NH, NSLOT * P], BF16)           # [d, h, t]

        # ================= projection phases =================
        with tc.tile_pool(name="xT_pool", bufs=1) as xT_pool, \
             tc.tile_pool(name="qw", bufs=1) as qw_pool, \
             tc.tile_pool(name="ph0ps", bufs=2, space="PSUM") as ps0:
            xT = xT_pool.tile([P, HB, NSLOT * P], BF16)       # [h%128, hb, t]
            wq_sb = qw_pool.tile([P, HB, NH * HD], BF16, name="wq_sb")
            # x arrives pre-transposed (host-side) as [p, hb, t] bf16
            for c in range(2):
                HBH = HB // 2
                WH = HBH * NSLOT * P
                nc.sync.dma_start(
                    xT[:, c * HBH:(c + 1) * HBH, :],
                    x_p[:, c * WH:(c + 1) * WH]
                    .rearrange("p (hb t) -> p hb t", hb=HBH))

            # ---- phase 1a: K/V proj + RoPE + per-slot staging + AllGather ----
            with tc.tile_pool(name="kvw", bufs=1) as kvw_pool, \
                 tc.tile_pool(name="kvstage", bufs=2) as kvstage, \
                 tc.tile_pool(name="ktps", bufs=2, space="PSUM") as ktps:
                rope.pool = kvstage
                wk_sb = kvw_pool.tile([P, HB, KVW], BF16, name="wk_sb")
                wv_sb = kvw_pool.tile([P, HB, KVW], BF16, name="wv_sb")
                for c in range(2):
                    HBH = HB // 2
                    WH = HBH * KVW
                    nc.sync.dma_start(
                        wk_sb[:, c * HBH:(c + 1) * HBH, :],
                        wk_p[:, c * WH:(c + 1) * WH]
                        .rearrange("p (hb n) -> p hb n", hb=HBH))
                    nc.sync.dma_start(
                        wv_sb[:, c * HBH:(c + 1) * HBH, :],
                        wv_p[:, c * WH:(c + 1) * WH]
                        .rearrange("p (hb n) -> p hb n", hb=HBH))
                for ap, prm in ((cosk_t, cosk_p), (sink_t, sink_p),
                                (cosq_t, cosq_p), (sinq_t, sinq_p)):
                    nc.sync.dma_start(ap[:], prm[:].rearrange("(s p) d -> p s d", p=P))
                nc.sync.dma_start(masks_t[:], masks_p[:].rearrange("n p d -> p n d"))
                for c in range(4):
                    HBQ = HB // 4
                    WQ4 = HBQ * NH * HD
                    nc.sync.dma_start(
                        wq_sb[:, c * HBQ:(c + 1) * HBQ, :],
                        wq_p[:, c * WQ4:(c + 1) * WQ4]
                        .rearrange("p (hb n) -> p hb n", hb=HBQ))

                for s in range(NSLOT):
                    pk = ps0.tile([P, KVW], FP32, name="pk", tag="pkv")
                    for hb in range(HB):
                        nc.tensor.matmul(pk[:], xT[:, hb, s * P:(s + 1) * P],
                                         wk_sb[:, hb, :],
                                         start=(hb == 0), stop=(hb == HB - 1))
                    kr = kvstage.tile([P, KVH, HD], FP32, name=f"k_rope{s}",
                                      tag=f"k_rope{s % 2}")
                    rope(nc.vector, kr, pk, cosk_t, sink_t, s, KVH)

                    pv = ps0.tile([P, KVW], FP32, name="pv", tag="pkv")
                    for hb in range(HB):
                        nc.tensor.matmul(pv[:], xT[:, hb, s * P:(s + 1) * P],
                                         wv_sb[:, hb, :],
                                         start=(hb == 0), stop=(hb == HB - 1))
                    vst = kvstage.tile([P, KVW], BF16, name=f"v_st{s}", tag="v_st")
                    nc.vector.tensor_copy(vst[:], pv[:])
                    agi, si = (ag_in1, s) if s < 2 else (ag_in2, s - 2)
                    nc.sync.dma_start(
                        agi[2 * SLOT_K + si * SLOT_V:
                            2 * SLOT_K + (si + 1) * SLOT_V]
                        .rearrange("(p w) -> p w", p=P),
                        vst[:].bitcast(FP32))

                    # transpose this slot's k and stage it (bf16)
                    pkt = ktps.tile([P, KVH * P], FP32, name="pkt", tag="pkt")
                    for g in range(KVH):
                        nc.tensor.transpose(pkt[:, g * P:(g + 1) * P],
                                            kr[:, g, :], ident_f32[:])
                    kst = kvstage.tile([P, KVH, P], BF16, name=f"k_st{s}",
                                       tag="k_st")
                    nc.vector.tensor_copy(
                        kst[:], pkt[:].rearrange("p (g t) -> p g t", t=P))
                    nc.sync.dma_start(
                        agi[si * SLOT_K:(si + 1) * SLOT_K]
                        .rearrange("(d g w) -> d g w", d=P, g=KVH),
                        kst[:].bitcast(FP32))
                    if s == 1:
                        nc.gpsimd.collective_compute(
                            "AllGather", mybir.AluOpType.bypass,
                            replica_groups=groups,
                            ins=[ag_in1[:]], outs=[ag_out1[:]])
                    elif s == 3:
                        nc.gpsimd.collective_compute(
                            "AllGather", mybir.AluOpType.bypass,
                            replica_groups=groups,
                            ins=[ag_in2[:]], outs=[ag_out2[:]])

            # ---- phase 1b: Q projection + RoPE + transpose to qT ----
            QC = 4  # heads per Wq chunk
            with tc.tile_pool(name="qstage", bufs=3) as qstage, \
                 tc.tile_pool(name="qps", bufs=2, space="PSUM") as qps, \
                 tc.tile_pool(name="qtps", bufs=2, space="PSUM") as qtps:
                rope.pool = qstage
                for hc in range(NH // QC):
                    q_rope = []
                    for s in range(NSLOT):
                        pq = qps.tile([P, QC * HD], FP32, name="pq", tag="pq")
                        for hb in range(HB):
                            nc.tensor.matmul(pq[:], xT[:, hb, s * P:(s + 1) * P],
                                             wq_sb[:, hb,
                                                   hc * QC * HD:(hc + 1) * QC * HD],
                                             start=(hb == 0), stop=(hb == HB - 1))
                        qr = qstage.tile([P, QC, HD], FP32, name=f"q_rope{s}",
                                         tag=f"q_rope{s % 2}")
                        rope(nc.vector, qr, pq, cosq_t, sinq_t, s, QC)
                        q_rope.append(qr)
                    for h in range(QC):
                        pqt = qtps.tile([P, NSLOT * P], FP32, name="pqt", tag="pqt")
                        for s in range(NSLOT):
                            nc.tensor.transpose(pqt[:, s * P:(s + 1) * P],
                                                q_rope[s][:, h, :], ident_f32[:])
                        nc.vector.tensor_copy(qT[:, hc * QC + h, :], pqt[:])

        # ================= gather + attention + interleaved Wo =================
        with tc.tile_pool(name="kv_pool", bufs=1) as kv_pool, \
             tc.tile_pool(name="wopool", bufs=1) as wopool:
            kT = kv_pool.tile([P, KVH, T], BF16)          # [d, g, t(batch)]
            v_all = kv_pool.tile([P, NBLK, KVW], BF16)    # [t%128, blk, (g d)]

            # block j was produced by in-group position pos=3-(j%4), slot s=j//4
            for j in range(NBLK):
                s, pos = j // 4, 3 - (j % 4)
                ago, si = (ag_out1, s) if s < 2 else (ag_out2, s - 2)
                nc.sync.dma_start(
                    v_all[:, j, :],
                    ago[pos, 2 * SLOT_K + si * SLOT_V:
                        2 * SLOT_K + (si + 1) * SLOT_V]
                    .rearrange("(p w) -> p w", p=P).bitcast(BF16))
                nc.sync.dma_start(
                    kT[:, :, j * P:(j + 1) * P],
                    ago[pos, si * SLOT_K:(si + 1) * SLOT_K]
                    .rearrange("(d g w) -> d g w", d=P, g=KVH).bitcast(BF16))

            wo_sb = wopool.tile([P, HB, HID], BF16, name="wo_sb")
            for c in range(4):
                HBQ = HB // 4
                WO4 = HBQ * HID
                nc.sync.dma_start(
                    wo_sb[:, c * HBQ:(c + 1) * HBQ, :],
                    wo_p[:, c * WO4:(c + 1) * WO4]
                    .rearrange("p (hb n) -> p hb n", hb=HBQ))

            with tc.tile_pool(name="ppool", bufs=3) as ppool, \
                 tc.tile_pool(name="astage", bufs=2) as astage, \
                 tc.tile_pool(name="ctxp", bufs=1) as ctxp, \
                 tc.tile_pool(name="ostage", bufs=3) as ostage, \
                 tc.tile_pool(name="scps", bufs=2, space="PSUM") as scps, \
                 tc.tile_pool(name="cps", bufs=2, space="PSUM") as cps, \
                 tc.tile_pool(name="rps", bufs=2, space="PSUM") as rps, \
                 tc.tile_pool(name="ops", bufs=2, space="PSUM") as ops:
                OC = HID // 4

                def wo_chunk(ws, oc, wctx):
                    po = ops.tile([P, OC], FP32, name="po", tag="po")
                    for wg in range(KVH):
                        for wh in range(GPQ):
                            hh = wg * GPQ + wh
                            nc.tensor.matmul(po[:], wctx[:, wg, wh, :],
                                             wo_sb[:, hh, oc * OC:(oc + 1) * OC],
                                             start=(hh == 0), stop=(hh == HB - 1))
                    ot = ostage.tile([P, OC], FP32, name="ot", tag="ot")
                    nc.vector.tensor_copy(ot[:], po[:])
                    nc.sync.dma_start(
                        out_p[ws * P:(ws + 1) * P, oc * OC:(oc + 1) * OC], ot[:])

                for s in range(NSLOT):
                    Es = E[s]
                    ctx_s = ctxp.tile([P, KVH, GPQ, P], BF16, name=f"ctx{s}",
                                      tag=f"ctx{s % 2}")
                    for g in range(KVH):
                        q_rhs = qT[:, g * GPQ:(g + 1) * GPQ, s * P:(s + 1) * P]
                        pctx = cps.tile([P, GPQ * P], FP32, name="pctx", tag="pctx")
                        prs = rps.tile([P, GPQ * P], FP32, name="prs", tag="prs")
                        pts = []
                        for kb in range(Es):
                            psc = scps.tile([P, GPQ * P], FP32, name="psc", tag="psc")
                            mi = mask_idx.get((s, kb))
                            nc.tensor.matmul(
                                psc[:], kT[:, g, kb * P:(kb + 1) * P],
                                q_rhs, start=True, stop=True)
                            if mi is not None:
                                psc3 = psc[:].rearrange("p (h t) -> p h t", t=P)
                                nc.vector.tensor_tensor(
                                    psc3, psc3,
                                    masks_t[:, mi, None, :]
                                    .to_broadcast((P, GPQ, P)),
                                    mybir.AluOpType.add)
                            pt = ppool.tile([P, GPQ * P], BF16, name="pt", tag="pt")
                            nc.scalar.activation(pt[:], psc[:], Exp)
                            nc.tensor.matmul(pctx[:],
                                             v_all[:, kb, g * HD:(g + 1) * HD],
                                             pt[:],
                                             start=(kb == 0), stop=(kb == Es - 1))
                            pts.append(pt)
                            if kb % 4 == 3:
                                pa = ppool.tile([P, GPQ * P], BF16,
                                                name="pa", tag="pa")
                                pb = ppool.tile([P, GPQ * P], BF16,
                                                name="pb", tag="pb")
                                nc.vector.tensor_tensor(pa[:], pts[-4][:],
                                                        pts[-3][:],
                                                        mybir.AluOpType.add)
                                nc.vector.tensor_tensor(pb[:], pa[:], pts[-2][:],
                                                        mybir.AluOpType.add)
                                nc.vector.tensor_tensor(pa[:], pb[:], pts[-1][:],
                                                        mybir.AluOpType.add)
                                nc.tensor.matmul(prs[:], ones_bf[:], pa[:],
                                                 start=(kb == 3),
                                                 stop=(kb == Es - 1))
                                pts = []
                        for i, pt in enumerate(pts):
                            nc.tensor.matmul(prs[:], ones_bf[:], pt[:],
                                             start=(Es < 4 and i == 0),
                                             stop=(i == len(pts) - 1))
                        rr = astage.tile([P, GPQ * P], FP32, name="rr", tag="rr")
                        nc.vector.reciprocal_approx_fast(rr[:], prs[:])
                        nc.vector.tensor_tensor(
                            ctx_s[:, g, :, :],
                            pctx[:].rearrange("p (h t) -> p h t", t=P),
                            rr[:].rearrange("p (h t) -> p h t", t=P),
                            mybir.AluOpType.mult)
                        # fill exp-wait gaps with the previous slot's Wo chunk
                        if s > 0:
                            wo_chunk(s - 1, g, ctx_prev)
                    ctx_prev = ctx_s
                for oc in range(4):
                    wo_chunk(NSLOT - 1, oc, ctx_prev)

    nc.compile()
    return nc


def _prep_inputs(hidden_states, attention_mask, cos, sin, Wq, Wk, Wv, Wo, P_list):
    hs = np.ascontiguousarray(np.asarray(hidden_states, dtype=np.float32))
    mask = np.asarray(attention_mask, dtype=np.float32).reshape(T, T)
    cos2 = np.asarray(cos, dtype=np.float32).reshape(T, HD)
    sin2 = np.asarray(sin, dtype=np.float32).reshape(T, HD)
    scale = np.float32(1.0 / np.sqrt(HD))

    def t3(s_):
        # rotate_half add trick: t3 = concat(sin[:, 64:], -sin[:, :64])
        return np.concatenate([s_[:, HD // 2:], -s_[:, :HD // 2]], axis=1)

    bf = ml_dtypes.bfloat16

    def wprep(w):
        w = np.asarray(w, dtype=np.float32).astype(bf)
        n = w.shape[1]
        return np.ascontiguousarray(
            w.reshape(HB, P, n).transpose(1, 0, 2).reshape(P, HB * n))

    wq = wprep(Wq)
    wk = wprep(Wk)
    wv = wprep(Wv)
    wo = wprep(Wo)

    in_maps = []
    for i in range(NC):
        b, pos = i // 4, i % 4
        js = [4 * s + 3 - pos for s in range(NSLOT)]
        take = lambda a: np.ascontiguousarray(
            np.concatenate([a[j * P:(j + 1) * P] for j in js], axis=0))
        m_tiles = [mask[js[s] * P:(js[s] + 1) * P, kb * P:(kb + 1) * P].T
                   for (s, kb) in P_list]
        if not m_tiles:
            m_tiles.append(np.zeros((P, P), np.float32))
        xc = take(hs[b])                         # [512, 2048] fp32
        xt = np.ascontiguousarray(
            xc.T.reshape(HB, P, NSLOT * P).transpose(1, 0, 2)
            .reshape(P, HB * NSLOT * P).astype(bf))
        in_maps.append({
            "x": xt,
            "wq": wq, "wk": wk, "wv": wv, "wo": wo,
            "cosq": take(cos2 * scale),
            "sinq3": take(t3(sin2 * scale)),
            "cosk": take(cos2),
            "sink3": take(t3(sin2)),
            "masks": np.stack(m_tiles).astype(bf),
        })
    return in_maps


_cache = {}


def kernel(hidden_states, attention_mask, cos, sin, Wq, Wk, Wv, Wo,
           _trace=False, _trace_kwargs=None):
    from concourse.bass_utils import run_bass_kernel_spmd

    E, P_list = _mask_plan(attention_mask)
    key = (tuple(E), tuple(P_list))
    if key not in _cache:
        _cache[key] = _build_program(E, P_list)
    nc = _cache[key]

    in_maps = _prep_inputs(hidden_states, attention_mask, cos, sin,
                           Wq, Wk, Wv, Wo, P_list)
    kwargs = dict(_trace_kwargs or {})
    if _trace:
        kwargs["trace"] = True
    res = run_bass_kernel_spmd(nc, in_maps, list(range(NC)), **kwargs)

    out = np.empty((B, T, HID), dtype=np.float32)
    for i in range(NC):
        b, pos = i // 4, i % 4
        o = res.results[i]["out"]
        for s in range(NSLOT):
            j = 4 * s + 3 - pos
            out[b, j * P:(j + 1) * P, :] = o[s * P:(s + 1) * P, :]
    kernel._last_result = res
    return out


# revision 20
# speedup vs baseline: 1.1083x; 1.0136x over previous
# Trainium2 Bass kernel for AvaAttention (GQA attention + RoPE + additive mask)
# B=2, T=2048, HID=2048, NH=16, KVH=4, HD=128, fp32 — 8 NeuronCores.
#
# Sharding: sequence-parallel. Core i (batch b=i//4, position p=i%4) owns
# q-blocks j = 4s+3-p of batch b, for slot s in 0..3. Projections are
# row-parallel (weights replicated, bf16), K/V exchanged with a SINGLE
# combined AllGather (bf16 payloads packed in a flat fp32 buffer) over
# each batch's 4 cores; attention + output projection stay local.
#
# v3 notes:
#  - Projections run in bf16 (bf16 xT + bf16 weights); RoPE in fp32 from
#    the fp32 PSUM projection result; q/k re-cast to bf16 on the
#    PSUM->SBUF eviction after their transposes.
#  - Scores are computed pre-transposed ([tk, (h tq)]): K block is the
#    stationary operand, 4 q-heads stream at once (N=512). No per-head
#    diag/transpose matmuls, no PSUM->bf16 CAST of probabilities.
#  - Softmax denominators via an all-ones stationary matmul accumulated
#    over kb; result is replicated across partitions so normalization is
#    one elementwise multiply fused into the ctx PSUM->SBUF eviction.
#    1/x via the fast custom-DVE reciprocal (plain reciprocal is ~3.4us
#    per tile and serialized the attention tail).
#  - Additive mask: one N=512 matmul per masked tile (lhsT = mask data in
#    natural [tq, tk], rhs = 4 identity blocks).
#  - Wo is bf16, fully resident in SBUF, and its matmuls interleave with
#    attention per-slot so the tensor engine stays busy to the end.
#  - exp without max-subtraction (safe at this score scale; masked
#    positions hit exp(S-1e9)=0).

import sys

for _p in ("/opt/trn_rl_repo", "/opt/pypackages"):
    if _p not in sys.path:
        sys.path.insert(0, _p)

import numpy as np
import ml_dtypes

B, T, HID = 2, 2048, 2048
NH, KVH, HD = 16, 4, 128
P = 128
NC = 8
NBLK = T // P          # 16 q-blocks per batch
NSLOT = 4              # blocks per core
GPQ = NH // KVH        # 4 q-heads per kv group
HB = HID // P          # 16 contraction subtiles
NEG_THRESH = -1.0e8
KVW = KVH * HD         # 512
SLOT_K = P * KVH * P // 2        # 32768 fp32 words: one slot's bf16 kT
SLOT_V = P * KVW // 2            # 32768 fp32 words: one slot's bf16 V
AG_K = NSLOT * SLOT_K            # 131072
AG_V = NSLOT * SLOT_V            # 131072


def _mask_plan(attention_mask):
    """Classify the additive mask per (j, kb) 128x128 tile.

    Returns (E, P_list): E[s] is the uniform k-extent (in blocks) for slot
    s; P_list is the ordered list of (s, kb) positions where a mask-add is
    applied (positions shared by every core; tile *data* is per-core).
    """
    m = np.asarray(attention_mask).reshape(T, T)
    nonzero = np.zeros((NBLK, NBLK), dtype=bool)
    live = np.zeros((NBLK, NBLK), dtype=bool)   # not fully masked
    for j in range(NBLK):
        for kb in range(NBLK):
            tile = m[j * P:(j + 1) * P, kb * P:(kb + 1) * P]
            nonzero[j, kb] = bool(np.any(tile != 0.0))
            live[j, kb] = bool(np.any(tile > NEG_THRESH))
    kmax = np.ones(NBLK, dtype=int)
    for j in range(NBLK):
        idx = np.nonzero(live[j])[0]
        if len(idx):
            kmax[j] = int(idx[-1]) + 1
    E = [int(max(kmax[4 * s + jj] for jj in range(4))) for s in range(NSLOT)]
    P_list = []
    for s in range(NSLOT):
        for kb in range(E[s]):
            if any(nonzero[4 * s + jj, kb] for jj in range(4)):
                P_list.append((s, kb))
    return E, P_list


def _build_program(E, P_list):
    import concourse.mybir as mybir
    import concourse.tile as tile
    from concourse import bacc
    from concourse.masks import make_identity
    from contextlib import ExitStack

    FP32 = mybir.dt.float32
    FP32R = mybir.dt.float32r
    BF16 = mybir.dt.bfloat16
    FP8 = mybir.dt.float8e4
    DR = mybir.MatmulPerfMode.DoubleRow
    Exp = mybir.ActivationFunctionType.Exp
    HALF = HD // 2

    nc = bacc.Bacc("TRN2", target_bir_lowering=False, num_devices=NC)

    x_p = nc.declare_dram_parameter("x", [P, HB * NSLOT * P], BF16, isOutput=False)
    wq_p = nc.declare_dram_parameter("wq", [P, HB * NH * HD], BF16, isOutput=False)
    wk_p = nc.declare_dram_parameter("wk", [P, HB * KVH * HD], BF16, isOutput=False)
    wv_p = nc.declare_dram_parameter("wv", [P, HB * KVH * HD], BF16, isOutput=False)
    wo_p = nc.declare_dram_parameter("wo", [P, HB * HID], BF16, isOutput=False)
    cosq_p = nc.declare_dram_parameter("cosq", [NSLOT * P, HD], FP32, isOutput=False)
    sinq_p = nc.declare_dram_parameter("sinq3", [NSLOT * P, HD], FP32, isOutput=False)
    cosk_p = nc.declare_dram_parameter("cosk", [NSLOT * P, HD], FP32, isOutput=False)
    sink_p = nc.declare_dram_parameter("sink3", [NSLOT * P, HD], FP32, isOutput=False)
    nmask = max(1, len(P_list))
    masks_p = nc.declare_dram_parameter("masks", [nmask, P, P], BF16, isOutput=False)
    out_p = nc.declare_dram_parameter("out", [NSLOT * P, HID], FP32, isOutput=True)

    HALF_AG = 2 * (SLOT_K + SLOT_V)
    ag_in1 = nc.dram_tensor("ag_in1", [HALF_AG], FP32)
    ag_out1 = nc.dram_tensor("ag_out1", [4, HALF_AG], FP32, addr_space="Local")
    ag_in2 = nc.dram_tensor("ag_in2", [HALF_AG], FP32)
    ag_out2 = nc.dram_tensor("ag_out2", [4, HALF_AG], FP32, addr_space="Local")
    groups = [[0, 1, 2, 3], [4, 5, 6, 7]]

    mask_idx = {sk: idx for idx, sk in enumerate(P_list)}

    def rope(engine, dst, src_ps, cos_t, sin_t, s, nh):
        """dst[t, h, d] = src*cos + rotate_half(src)*sin, natural layout."""
        src3 = src_ps[:].rearrange("p (h d) -> p h d", d=HD)
        cst = rope.pool.tile([P, nh, HD], FP32, name="rope_c", tag="rope_c")
        engine.tensor_tensor(dst[:], src3,
                             cos_t[:, s, None, :].to_broadcast((P, nh, HD)),
                             mybir.AluOpType.mult)
        engine.tensor_tensor(cst[:], src3,
                             sin_t[:, s, None, :].to_broadcast((P, nh, HD)),
                             mybir.AluOpType.mult)
        engine.tensor_tensor(dst[:, :, HALF:], dst[:, :, HALF:],
                             cst[:, :, :HALF], mybir.AluOpType.add)
        engine.tensor_tensor(dst[:, :, :HALF], dst[:, :, :HALF],
                             cst[:, :, HALF:], mybir.AluOpType.add)

    with tile.TileContext(nc) as tc, ExitStack() as top:
        const = top.enter_context(tc.tile_pool(name="const", bufs=1))
        ident_f32 = const.tile([P, P], FP32)
        make_identity(nc, ident_f32[:])
        ones_bf = const.tile([P, P], BF16)
        nc.gpsimd.memset(ones_bf[:], 1.0)

        cosq_t = const.tile([P, NSLOT, HD], FP32)
        sinq_t = const.tile([P, NSLOT, HD], FP32)
        cosk_t = const.tile([P, NSLOT, HD], FP32)
        sink_t = const.tile([P, NSLOT, HD], FP32)
        masks_t = const.tile([P, nmask, P], BF16)

        qT_pool = top.enter_context(tc.tile_pool(name="qT_pool", bufs=1))
        qT = qT_pool.tile([P, NH, NSLOT * P], BF16)           # [d, h, t]

        # ================= projection phases =================
        with tc.tile_pool(name="xT_pool", bufs=1) as xT_pool, \
             tc.tile_pool(name="qw", bufs=1) as qw_pool, \
             tc.tile_pool(name="ph0ps", bufs=2, space="PSUM") as ps0:
            xT = xT_pool.tile([P, HB, NSLOT * P], BF16)       # [h%128, hb, t]
            wq_sb = qw_pool.tile([P, HB, NH * HD], BF16, name="wq_sb")
            # x arrives pre-transposed (host-side) as [p, hb, t] bf16
            for c in range(4):
                HBH = HB // 4
                WH = HBH * NSLOT * P
                nc.sync.dma_start(
                    xT[:, c * HBH:(c + 1) * HBH, :],
                    x_p[:, c * WH:(c + 1) * WH]
                    .rearrange("p (hb t) -> p hb t", hb=HBH))

            # ---- phase 1a: K/V proj + RoPE + per-slot staging + AllGather ----
            with tc.tile_pool(name="kvw", bufs=1) as kvw_pool, \
                 tc.tile_pool(name="kvstage", bufs=2) as kvstage, \
                 tc.tile_pool(name="ktps", bufs=2, space="PSUM") as ktps:
                rope.pool = kvstage
                wk_sb = kvw_pool.tile([P, HB, KVW], BF16, name="wk_sb")
                wv_sb = kvw_pool.tile([P, HB, KVW], BF16, name="wv_sb")
                for c in range(2):
                    HBH = HB // 2
                    WH = HBH * KVW
                    nc.sync.dma_start(
                        wk_sb[:, c * HBH:(c + 1) * HBH, :],
                        wk_p[:, c * WH:(c + 1) * WH]
                        .rearrange("p (hb n) -> p hb n", hb=HBH))
                    nc.sync.dma_start(
                        wv_sb[:, c * HBH:(c + 1) * HBH, :],
                        wv_p[:, c * WH:(c + 1) * WH]
                        .rearrange("p (hb n) -> p hb n", hb=HBH))
                for ap, prm in ((cosk_t, cosk_p), (sink_t, sink_p),
                                (cosq_t, cosq_p), (sinq_t, sinq_p)):
                    nc.sync.dma_start(ap[:], prm[:].rearrange("(s p) d -> p s d", p=P))
                nc.sync.dma_start(masks_t[:], masks_p[:].rearrange("n p d -> p n d"))
                for c in range(4):
                    HBQ = HB // 4
                    WQ4 = HBQ * NH * HD
                    nc.sync.dma_start(
                        wq_sb[:, c * HBQ:(c + 1) * HBQ, :],
                        wq_p[:, c * WQ4:(c + 1) * WQ4]
                        .rearrange("p (hb n) -> p hb n", hb=HBQ))

                for s in range(NSLOT):
                    pk = ps0.tile([P, KVW], FP32, name="pk", tag="pkv")
                    for hb in range(HB):
                        nc.tensor.matmul(pk[:], xT[:, hb, s * P:(s + 1) * P],
                                         wk_sb[:, hb, :],
                                         start=(hb == 0), stop=(hb == HB - 1))
                    kr = kvstage.tile([P, KVH, HD], FP32, name=f"k_rope{s}",
                                      tag=f"k_rope{s % 2}")
                    rope(nc.vector, kr, pk, cosk_t, sink_t, s, KVH)

                    pv = ps0.tile([P, KVW], FP32, name="pv", tag="pkv")
                    for hb in range(HB):
                        nc.tensor.matmul(pv[:], xT[:, hb, s * P:(s + 1) * P],
                                         wv_sb[:, hb, :],
                                         start=(hb == 0), stop=(hb == HB - 1))
                    vst = kvstage.tile([P, KVW], BF16, name=f"v_st{s}", tag="v_st")
                    nc.vector.tensor_copy(vst[:], pv[:])
                    agi, si = (ag_in1, s) if s < 2 else (ag_in2, s - 2)
                    nc.sync.dma_start(
                        agi[2 * SLOT_K + si * SLOT_V:
                            2 * SLOT_K + (si + 1) * SLOT_V]
                        .rearrange("(p w) -> p w", p=P),
                        vst[:].bitcast(FP32))

                    # transpose this slot's k and stage it (bf16)
                    pkt = ktps.tile([P, KVH * P], FP32, name="pkt", tag="pkt")
                    for g in range(KVH):
                        nc.tensor.transpose(pkt[:, g * P:(g + 1) * P],
                                            kr[:, g, :], ident_f32[:])
                    kst = kvstage.tile([P, KVH, P], BF16, name=f"k_st{s}",
                                       tag="k_st")
                    nc.vector.tensor_copy(
                        kst[:], pkt[:].rearrange("p (g t) -> p g t", t=P))
                    nc.sync.dma_start(
                        agi[si * SLOT_K:(si + 1) * SLOT_K]
                        .rearrange("(d g w) -> d g w", d=P, g=KVH),
                        kst[:].bitcast(FP32))
                    if s == 1:
                        nc.gpsimd.collective_compute(
                            "AllGather", mybir.AluOpType.bypass,
                            replica_groups=groups,
                            ins=[ag_in1[:]], outs=[ag_out1[:]])
                    elif s == 3:
                        nc.gpsimd.collective_compute(
                            "AllGather", mybir.AluOpType.bypass,
                            replica_groups=groups,
                            ins=[ag_in2[:]], outs=[ag_out2[:]])

            # ---- phase 1b: Q projection + RoPE + transpose to qT ----
            QC = 4  # heads per Wq chunk
            with tc.tile_pool(name="qstage", bufs=3) as qstage, \
                 tc.tile_pool(name="qps", bufs=2, space="PSUM") as qps, \
                 tc.tile_pool(name="qtps", bufs=2, space="PSUM") as qtps:
                rope.pool = qstage
                for hc in range(NH // QC):
                    q_rope = []
                    for s in range(NSLOT):
                        pq = qps.tile([P, QC * HD], FP32, name="pq", tag="pq")
                        for hb in range(HB):
                            nc.tensor.matmul(pq[:], xT[:, hb, s * P:(s + 1) * P],
                                             wq_sb[:, hb,
                                                   hc * QC * HD:(hc + 1) * QC * HD],
                                             start=(hb == 0), stop=(hb == HB - 1))
                        qr = qstage.tile([P, QC, HD], FP32, name=f"q_rope{s}",
                                         tag=f"q_rope{s % 2}")
                        rope(nc.vector, qr, pq, cosq_t, sinq_t, s, QC)
                        q_rope.append(qr)
                    for h in range(QC):
                        pqt = qtps.tile([P, NSLOT * P], FP32, name="pqt", tag="pqt")
                        for s in range(NSLOT):
                            nc.tensor.transpose(pqt[:, s * P:(s + 1) * P],
                                                q_rope[s][:, h, :], ident_f32[:])
                        nc.vector.tensor_copy(qT[:, hc * QC + h, :], pqt[:])

        # ================= gather + attention + interleaved Wo =================
        with tc.tile_pool(name="kv_pool", bufs=1) as kv_pool, \
             tc.tile_pool(name="wopool", bufs=1) as wopool:
            kT = kv_pool.tile([P, KVH, T], BF16)          # [d, g, t(batch)]
            v_all = kv_pool.tile([P, NBLK, KVW], BF16)    # [t%128, blk, (g d)]

            # block j was produced by in-group position pos=3-(j%4), slot s=j//4
            for j in range(NBLK):
                s, pos = j // 4, 3 - (j % 4)
                ago, si = (ag_out1, s) if s < 2 else (ag_out2, s - 2)
                nc.sync.dma_start(
                    v_all[:, j, :],
                    ago[pos, 2 * SLOT_K + si * SLOT_V:
                        2 * SLOT_K + (si + 1) * SLOT_V]
                    .rearrange("(p w) -> p w", p=P).bitcast(BF16))
                nc.sync.dma_start(
                    kT[:, :, j * P:(j + 1) * P],
                    ago[pos, si * SLOT_K:(si + 1) * SLOT_K]
                    .rearrange("(d g w) -> d g w", d=P, g=KVH).bitcast(BF16))

            wo_sb = wopool.tile([P, HB, HID], BF16, name="wo_sb")
            for c in range(4):
                HBQ = HB // 4
                WO4 = HBQ * HID
                nc.sync.dma_start(
                    wo_sb[:, c * HBQ:(c + 1) * HBQ, :],
                    wo_p[:, c * WO4:(c + 1) * WO4]
                    .rearrange("p (hb n) -> p hb n", hb=HBQ))

            with tc.tile_pool(name="ppool", bufs=3) as ppool, \
                 tc.tile_pool(name="astage", bufs=2) as astage, \
                 tc.tile_pool(name="ctxp", bufs=1) as ctxp, \
                 tc.tile_pool(name="ostage", bufs=3) as ostage, \
                 tc.tile_pool(name="scps", bufs=3, space="PSUM") as scps, \
                 tc.tile_pool(name="cps", bufs=2, space="PSUM") as cps, \
                 tc.tile_pool(name="rps", bufs=1, space="PSUM") as rps, \
                 tc.tile_pool(name="ops", bufs=2, space="PSUM") as ops:
                OC = HID // 4

                def wo_chunk(ws, oc, wctx):
                    po = ops.tile([P, OC], FP32, name="po", tag="po")
                    for wg in range(KVH):
                        for wh in range(GPQ):
                            hh = wg * GPQ + wh
                            nc.tensor.matmul(po[:], wctx[:, wg, wh, :],
                                             wo_sb[:, hh, oc * OC:(oc + 1) * OC],
                                             start=(hh == 0), stop=(hh == HB - 1))
                    ot = ostage.tile([P, OC], FP32, name="ot", tag="ot")
                    nc.vector.tensor_copy(ot[:], po[:])
                    nc.sync.dma_start(
                        out_p[ws * P:(ws + 1) * P, oc * OC:(oc + 1) * OC], ot[:])

                for s in range(NSLOT):
                    Es = E[s]
                    ctx_s = ctxp.tile([P, KVH, GPQ, P], BF16, name=f"ctx{s}",
                                      tag=f"ctx{s % 2}")
                    for g in range(KVH):
                        q_rhs = qT[:, g * GPQ:(g + 1) * GPQ, s * P:(s + 1) * P]
                        pctx = cps.tile([P, GPQ * P], FP32, name="pctx", tag="pctx")
                        prs = rps.tile([P, GPQ * P], FP32, name="prs", tag="prs")
                        pts = []
                        for kb in range(Es):
                            psc = scps.tile([P, GPQ * P], FP32, name="psc", tag="psc")
                            mi = mask_idx.get((s, kb))
                            nc.tensor.matmul(
                                psc[:], kT[:, g, kb * P:(kb + 1) * P],
                                q_rhs, start=True, stop=True)
                            if mi is not None:
                                psc3 = psc[:].rearrange("p (h t) -> p h t", t=P)
                                nc.vector.tensor_tensor(
                                    psc3, psc3,
                                    masks_t[:, mi, None, :]
                                    .to_broadcast((P, GPQ, P)),
                                    mybir.AluOpType.add)
                            pt = ppool.tile([P, GPQ * P], BF16, name="pt", tag="pt")
                            nc.scalar.activation(pt[:], psc[:], Exp)
                            nc.tensor.matmul(pctx[:],
                                             v_all[:, kb, g * HD:(g + 1) * HD],
                                             pt[:],
                                             start=(kb == 0), stop=(kb == Es - 1))
                            pts.append(pt)
                            if kb % 4 == 3:
                                pa = ppool.tile([P, GPQ * P], BF16,
                                                name="pa", tag="pa")
                                pb = ppool.tile([P, GPQ * P], BF16,
                                                name="pb", tag="pb")
                                nc.vector.tensor_tensor(pa[:], pts[-4][:],
                                                        pts[-3][:],
                                                        mybir.AluOpType.add)
                                nc.vector.tensor_tensor(pb[:], pa[:], pts[-2][:],
                                                        mybir.AluOpType.add)
                                nc.vector.tensor_tensor(pa[:], pb[:], pts[-1][:],
                                                        mybir.AluOpType.add)
                                nc.tensor.matmul(prs[:], ones_bf[:], pa[:],
                                                 start=(kb == 3),
                                                 stop=(kb == Es - 1))
                                pts = []
                        for i, pt in enumerate(pts):
                            nc.tensor.matmul(prs[:], ones_bf[:], pt[:],
                                             start=(Es < 4 and i == 0),
                                             stop=(i == len(pts) - 1))
                        rr = astage.tile([P, GPQ * P], FP32, name="rr", tag="rr")
                        nc.vector.reciprocal_approx_fast(rr[:], prs[:])
                        nc.vector.tensor_tensor(
                            ctx_s[:, g, :, :],
                            pctx[:].rearrange("p (h t) -> p h t", t=P),
                            rr[:].rearrange("p (h t) -> p h t", t=P),
                            mybir.AluOpType.mult)
                        # fill exp-wait gaps with the previous slot's Wo chunk
                        if s > 0:
                            wo_chunk(s - 1, g, ctx_prev)
                    ctx_prev = ctx_s
                for oc in range(4):
                    wo_chunk(NSLOT - 1, oc, ctx_prev)

    nc.compile()
    return nc


def _prep_inputs(hidden_states, attention_mask, cos, sin, Wq, Wk, Wv, Wo, P_list):
    hs = np.ascontiguousarray(np.asarray(hidden_states, dtype=np.float32))
    mask = np.asarray(attention_mask, dtype=np.float32).reshape(T, T)
    cos2 = np.asarray(cos, dtype=np.float32).reshape(T, HD)
    sin2 = np.asarray(sin, dtype=np.float32).reshape(T, HD)
    scale = np.float32(1.0 / np.sqrt(HD))

    def t3(s_):
        # rotate_half add trick: t3 = concat(sin[:, 64:], -sin[:, :64])
        return np.concatenate([s_[:, HD // 2:], -s_[:, :HD // 2]], axis=1)

    bf = ml_dtypes.bfloat16

    def wprep(w):
        w = np.asarray(w, dtype=np.float32).astype(bf)
        n = w.shape[1]
        return np.ascontiguousarray(
            w.reshape(HB, P, n).transpose(1, 0, 2).reshape(P, HB * n))

    wq = wprep(Wq)
    wk = wprep(Wk)
    wv = wprep(Wv)
    wo = wprep(Wo)

    in_maps = []
    for i in range(NC):
        b, pos = i // 4, i % 4
        js = [4 * s + 3 - pos for s in range(NSLOT)]
        take = lambda a: np.ascontiguousarray(
            np.concatenate([a[j * P:(j + 1) * P] for j in js], axis=0))
        m_tiles = [mask[js[s] * P:(js[s] + 1) * P, kb * P:(kb + 1) * P].T
                   for (s, kb) in P_list]
        if not m_tiles:
            m_tiles.append(np.zeros((P, P), np.float32))
        xc = take(hs[b])                         # [512, 2048] fp32
        xt = np.ascontiguousarray(
            xc.T.reshape(HB, P, NSLOT * P).transpose(1, 0, 2)
            .reshape(P, HB * NSLOT * P).astype(bf))
        in_maps.append({
            "x": xt,
            "wq": wq, "wk": wk, "wv": wv, "wo": wo,
            "cosq": take(cos2 * scale),
            "sinq3": take(t3(sin2 * scale)),
            "cosk": take(cos2),
            "sink3": take(t3(sin2)),
            "masks": np.stack(m_tiles).astype(bf),
        })
    return in_maps


_cache = {}


def kernel(hidden_states, attention_mask, cos, sin, Wq, Wk, Wv, Wo,
           _trace=False, _trace_kwargs=None):
    from concourse.bass_utils import run_bass_kernel_spmd

    E, P_list = _mask_plan(attention_mask)
    key = (tuple(E), tuple(P_list))
    if key not in _cache:
        _cache[key] = _build_program(E, P_list)
    nc = _cache[key]

    in_maps = _prep_inputs(hidden_states, attention_mask, cos, sin,
                           Wq, Wk, Wv, Wo, P_list)
    kwargs = dict(_trace_kwargs or {})
    if _trace:
        kwargs["trace"] = True
    res = run_bass_kernel_spmd(nc, in_maps, list(range(NC)), **kwargs)

    out = np.empty((B, T, HID), dtype=np.float32)
    for i in range(NC):
        b, pos = i // 4, i % 4
        o = res.results[i]["out"]
        for s in range(NSLOT):
            j = 4 * s + 3 - pos
            out[b, j * P:(j + 1) * P, :] = o[s * P:(s + 1) * P, :]
    kernel._last_result = res
    return out


# revision 22
# speedup vs baseline: 1.1140x; 1.0051x over previous
# Trainium2 Bass kernel for AvaAttention (GQA attention + RoPE + additive mask)
# B=2, T=2048, HID=2048, NH=16, KVH=4, HD=128, fp32 — 8 NeuronCores.
#
# Sharding: sequence-parallel. Core i (batch b=i//4, position p=i%4) owns
# q-blocks j = 4s+3-p of batch b, for slot s in 0..3. Projections are
# row-parallel (weights replicated, bf16), K/V exchanged with a SINGLE
# combined AllGather (bf16 payloads packed in a flat fp32 buffer) over
# each batch's 4 cores; attention + output projection stay local.
#
# v3 notes:
#  - Projections run in bf16 (bf16 xT + bf16 weights); RoPE in fp32 from
#    the fp32 PSUM projection result; q/k re-cast to bf16 on the
#    PSUM->SBUF eviction after their transposes.
#  - Scores are computed pre-transposed ([tk, (h tq)]): K block is the
#    stationary operand, 4 q-heads stream at once (N=512). No per-head
#    diag/transpose matmuls, no PSUM->bf16 CAST of probabilities.
#  - Softmax denominators via an all-ones stationary matmul accumulated
#    over kb; result is replicated across partitions so normalization is
#    one elementwise multiply fused into the ctx PSUM->SBUF eviction.
#    1/x via the fast custom-DVE reciprocal (plain reciprocal is ~3.4us
#    per tile and serialized the attention tail).
#  - Additive mask: one N=512 matmul per masked tile (lhsT = mask data in
#    natural [tq, tk], rhs = 4 identity blocks).
#  - Wo is bf16, fully resident in SBUF, and its matmuls interleave with
#    attention per-slot so the tensor engine stays busy to the end.
#  - exp without max-subtraction (safe at this score scale; masked
#    positions hit exp(S-1e9)=0).

import sys

for _p in ("/opt/trn_rl_repo", "/opt/pypackages"):
    if _p not in sys.path:
        sys.path.insert(0, _p)

import numpy as np
import ml_dtypes

B, T, HID = 2, 2048, 2048
NH, KVH, HD = 16, 4, 128
P = 128
NC = 8
NBLK = T // P          # 16 q-blocks per batch
NSLOT = 4              # blocks per core
GPQ = NH // KVH        # 4 q-heads per kv group
HB = HID // P          # 16 contraction subtiles
NEG_THRESH = -1.0e8
KVW = KVH * HD         # 512
SLOT_K = P * KVH * P // 2        # 32768 fp32 words: one slot's bf16 kT
SLOT_V = P * KVW // 2            # 32768 fp32 words: one slot's bf16 V
AG_K = NSLOT * SLOT_K            # 131072
AG_V = NSLOT * SLOT_V            # 131072


def _mask_plan(attention_mask):
    """Classify the additive mask per (j, kb) 128x128 tile.

    Returns (E, P_list): E[s] is the uniform k-extent (in blocks) for slot
    s; P_list is the ordered list of (s, kb) positions where a mask-add is
    applied (positions shared by every core; tile *data* is per-core).
    """
    m = np.asarray(attention_mask).reshape(T, T)
    nonzero = np.zeros((NBLK, NBLK), dtype=bool)
    live = np.zeros((NBLK, NBLK), dtype=bool)   # not fully masked
    for j in range(NBLK):
        for kb in range(NBLK):
            tile = m[j * P:(j + 1) * P, kb * P:(kb + 1) * P]
            nonzero[j, kb] = bool(np.any(tile != 0.0))
            live[j, kb] = bool(np.any(tile > NEG_THRESH))
    kmax = np.ones(NBLK, dtype=int)
    for j in range(NBLK):
        idx = np.nonzero(live[j])[0]
        if len(idx):
            kmax[j] = int(idx[-1]) + 1
    E = [int(max(kmax[4 * s + jj] for jj in range(4))) for s in range(NSLOT)]
    P_list = []
    for s in range(NSLOT):
        for kb in range(E[s]):
            if any(nonzero[4 * s + jj, kb] for jj in range(4)):
                P_list.append((s, kb))
    return E, P_list


def _build_program(E, P_list):
    import concourse.mybir as mybir
    import concourse.tile as tile
    from concourse import bacc
    from concourse.masks import make_identity
    from contextlib import ExitStack

    FP32 = mybir.dt.float32
    FP32R = mybir.dt.float32r
    BF16 = mybir.dt.bfloat16
    FP8 = mybir.dt.float8e4
    DR = mybir.MatmulPerfMode.DoubleRow
    Exp = mybir.ActivationFunctionType.Exp
    HALF = HD // 2

    nc = bacc.Bacc("TRN2", target_bir_lowering=False, num_devices=NC)

    x_p = nc.declare_dram_parameter("x", [P, HB * NSLOT * P], BF16, isOutput=False)
    wq_p = nc.declare_dram_parameter("wq", [P, HB * NH * HD], BF16, isOutput=False)
    wk_p = nc.declare_dram_parameter("wk", [P, HB * KVH * HD], BF16, isOutput=False)
    wv_p = nc.declare_dram_parameter("wv", [P, HB * KVH * HD], BF16, isOutput=False)
    wo_p = nc.declare_dram_parameter("wo", [P, HB * HID], BF16, isOutput=False)
    cosq_p = nc.declare_dram_parameter("cosq", [NSLOT * P, HD], FP32, isOutput=False)
    sinq_p = nc.declare_dram_parameter("sinq3", [NSLOT * P, HD], FP32, isOutput=False)
    cosk_p = nc.declare_dram_parameter("cosk", [NSLOT * P, HD], FP32, isOutput=False)
    sink_p = nc.declare_dram_parameter("sink3", [NSLOT * P, HD], FP32, isOutput=False)
    nmask = max(1, len(P_list))
    masks_p = nc.declare_dram_parameter("masks", [nmask, P, P], BF16, isOutput=False)
    out_p = nc.declare_dram_parameter("out", [NSLOT * P, HID], FP32, isOutput=True)

    HALF_AG = 2 * (SLOT_K + SLOT_V)
    ag_in1 = nc.dram_tensor("ag_in1", [HALF_AG], FP32)
    ag_out1 = nc.dram_tensor("ag_out1", [4, HALF_AG], FP32, addr_space="Local")
    ag_in2 = nc.dram_tensor("ag_in2", [HALF_AG], FP32)
    ag_out2 = nc.dram_tensor("ag_out2", [4, HALF_AG], FP32, addr_space="Local")
    groups = [[0, 1, 2, 3], [4, 5, 6, 7]]

    mask_idx = {sk: idx for idx, sk in enumerate(P_list)}

    def rope(engine, dst, src_ps, cos_t, sin_t, s, nh):
        """dst[t, h, d] = src*cos + rotate_half(src)*sin, natural layout."""
        src3 = src_ps[:].rearrange("p (h d) -> p h d", d=HD)
        cst = rope.pool.tile([P, nh, HD], FP32, name="rope_c", tag="rope_c")
        engine.tensor_tensor(dst[:], src3,
                             cos_t[:, s, None, :].to_broadcast((P, nh, HD)),
                             mybir.AluOpType.mult)
        engine.tensor_tensor(cst[:], src3,
                             sin_t[:, s, None, :].to_broadcast((P, nh, HD)),
                             mybir.AluOpType.mult)
        engine.tensor_tensor(dst[:, :, HALF:], dst[:, :, HALF:],
                             cst[:, :, :HALF], mybir.AluOpType.add)
        engine.tensor_tensor(dst[:, :, :HALF], dst[:, :, :HALF],
                             cst[:, :, HALF:], mybir.AluOpType.add)

    with tile.TileContext(nc) as tc, ExitStack() as top:
        const = top.enter_context(tc.tile_pool(name="const", bufs=1))
        ident_f32 = const.tile([P, P], FP32)
        make_identity(nc, ident_f32[:])
        ones_bf = const.tile([P, P], BF16)
        nc.gpsimd.memset(ones_bf[:], 1.0)

        cosq_t = const.tile([P, NSLOT, HD], FP32)
        sinq_t = const.tile([P, NSLOT, HD], FP32)
        cosk_t = const.tile([P, NSLOT, HD], FP32)
        sink_t = const.tile([P, NSLOT, HD], FP32)
        masks_t = const.tile([P, nmask, P], BF16)

        qT_pool = top.enter_context(tc.tile_pool(name="qT_pool", bufs=1))
        qT = qT_pool.tile([P, NH, NSLOT * P], BF16)           # [d, h, t]

        # ================= projection phases =================
        with tc.tile_pool(name="xT_pool", bufs=1) as xT_pool, \
             tc.tile_pool(name="qw", bufs=1) as qw_pool, \
             tc.tile_pool(name="ph0ps", bufs=2, space="PSUM") as ps0:
            xT = xT_pool.tile([P, HB, NSLOT * P], BF16)       # [h%128, hb, t]
            wq_sb = qw_pool.tile([P, HB, NH * HD], BF16, name="wq_sb")
            # x arrives pre-transposed (host-side) as [p, hb, t] bf16
            for c in range(4):
                HBH = HB // 4
                WH = HBH * NSLOT * P
                nc.sync.dma_start(
                    xT[:, c * HBH:(c + 1) * HBH, :],
                    x_p[:, c * WH:(c + 1) * WH]
                    .rearrange("p (hb t) -> p hb t", hb=HBH))

            # ---- phase 1a: K/V proj + RoPE + per-slot staging + AllGather ----
            with tc.tile_pool(name="kvw", bufs=1) as kvw_pool, \
                 tc.tile_pool(name="kvstage", bufs=2) as kvstage, \
                 tc.tile_pool(name="ktps", bufs=2, space="PSUM") as ktps:
                rope.pool = kvstage
                wk_sb = kvw_pool.tile([P, HB, KVW], BF16, name="wk_sb")
                wv_sb = kvw_pool.tile([P, HB, KVW], BF16, name="wv_sb")
                for c in range(2):
                    HBH = HB // 2
                    WH = HBH * KVW
                    nc.sync.dma_start(
                        wk_sb[:, c * HBH:(c + 1) * HBH, :],
                        wk_p[:, c * WH:(c + 1) * WH]
                        .rearrange("p (hb n) -> p hb n", hb=HBH))
                    nc.sync.dma_start(
                        wv_sb[:, c * HBH:(c + 1) * HBH, :],
                        wv_p[:, c * WH:(c + 1) * WH]
                        .rearrange("p (hb n) -> p hb n", hb=HBH))
                for ap, prm in ((cosk_t, cosk_p), (sink_t, sink_p),
                                (cosq_t, cosq_p), (sinq_t, sinq_p)):
                    nc.sync.dma_start(ap[:], prm[:].rearrange("(s p) d -> p s d", p=P))
                nc.sync.dma_start(masks_t[:], masks_p[:].rearrange("n p d -> p n d"))
                for c in range(8):
                    HBO = HB // 8
                    WQ8 = HBO * NH * HD
                    nc.sync.dma_start(
                        wq_sb[:, c * HBO:(c + 1) * HBO, :],
                        wq_p[:, c * WQ8:(c + 1) * WQ8]
                        .rearrange("p (hb n) -> p hb n", hb=HBO))

                for s in range(NSLOT):
                    pk = ps0.tile([P, KVW], FP32, name="pk", tag="pkv")
                    for hb in range(HB):
                        nc.tensor.matmul(pk[:], xT[:, hb, s * P:(s + 1) * P],
                                         wk_sb[:, hb, :],
                                         start=(hb == 0), stop=(hb == HB - 1))
                    kr = kvstage.tile([P, KVH, HD], FP32, name=f"k_rope{s}",
                                      tag=f"k_rope{s % 2}")
                    rope(nc.vector, kr, pk, cosk_t, sink_t, s, KVH)

                    pv = ps0.tile([P, KVW], FP32, name="pv", tag="pkv")
                    for hb in range(HB):
                        nc.tensor.matmul(pv[:], xT[:, hb, s * P:(s + 1) * P],
                                         wv_sb[:, hb, :],
                                         start=(hb == 0), stop=(hb == HB - 1))
                    vst = kvstage.tile([P, KVW], BF16, name=f"v_st{s}", tag="v_st")
                    nc.vector.tensor_copy(vst[:], pv[:])
                    agi, si = (ag_in1, s) if s < 2 else (ag_in2, s - 2)
                    nc.sync.dma_start(
                        agi[2 * SLOT_K + si * SLOT_V:
                            2 * SLOT_K + (si + 1) * SLOT_V]
                        .rearrange("(p w) -> p w", p=P),
                        vst[:].bitcast(FP32))

                    # transpose this slot's k and stage it (bf16)
                    pkt = ktps.tile([P, KVH * P], FP32, name="pkt", tag="pkt")
                    for g in range(KVH):
                        nc.tensor.transpose(pkt[:, g * P:(g + 1) * P],
                                            kr[:, g, :], ident_f32[:])
                    kst = kvstage.tile([P, KVH, P], BF16, name=f"k_st{s}",
                                       tag="k_st")
                    nc.vector.tensor_copy(
                        kst[:], pkt[:].rearrange("p (g t) -> p g t", t=P))
                    nc.sync.dma_start(
                        agi[si * SLOT_K:(si + 1) * SLOT_K]
                        .rearrange("(d g w) -> d g w", d=P, g=KVH),
                        kst[:].bitcast(FP32))
                    if s == 1:
                        nc.gpsimd.collective_compute(
                            "AllGather", mybir.AluOpType.bypass,
                            replica_groups=groups,
                            ins=[ag_in1[:]], outs=[ag_out1[:]])
                    elif s == 3:
                        nc.gpsimd.collective_compute(
                            "AllGather", mybir.AluOpType.bypass,
                            replica_groups=groups,
                            ins=[ag_in2[:]], outs=[ag_out2[:]])

            # ---- phase 1b: Q projection + RoPE + transpose to qT ----
            QC = 4  # heads per Wq chunk
            with tc.tile_pool(name="qstage", bufs=3) as qstage, \
                 tc.tile_pool(name="qps", bufs=2, space="PSUM") as qps, \
                 tc.tile_pool(name="qtps", bufs=2, space="PSUM") as qtps:
                rope.pool = qstage
                for hc in range(NH // QC):
                    q_rope = []
                    for s in range(NSLOT):
                        pq = qps.tile([P, QC * HD], FP32, name="pq", tag="pq")
                        for hb in range(HB):
                            nc.tensor.matmul(pq[:], xT[:, hb, s * P:(s + 1) * P],
                                             wq_sb[:, hb,
                                                   hc * QC * HD:(hc + 1) * QC * HD],
                                             start=(hb == 0), stop=(hb == HB - 1))
                        qr = qstage.tile([P, QC, HD], FP32, name=f"q_rope{s}",
                                         tag=f"q_rope{s % 2}")
                        rope(nc.vector, qr, pq, cosq_t, sinq_t, s, QC)
                        q_rope.append(qr)
                    for h in range(QC):
                        pqt = qtps.tile([P, NSLOT * P], FP32, name="pqt", tag="pqt")
                        for s in range(NSLOT):
                            nc.tensor.transpose(pqt[:, s * P:(s + 1) * P],
                                                q_rope[s][:, h, :], ident_f32[:])
                        nc.vector.tensor_copy(qT[:, hc * QC + h, :], pqt[:])

        # ================= gather + attention + interleaved Wo =================
        with tc.tile_pool(name="kv_pool", bufs=1) as kv_pool, \
             tc.tile_pool(name="wopool", bufs=1) as wopool:
            kT = kv_pool.tile([P, KVH, T], BF16)          # [d, g, t(batch)]
            v_all = kv_pool.tile([P, NBLK, KVW], BF16)    # [t%128, blk, (g d)]

            # block j = 4s + (3-pos); gather both slots of a group per DMA
            v_view = v_all[:].rearrange("p (s r) w -> p s r w", r=4)
            for gi, ago in ((0, ag_out1), (1, ag_out2)):
                for pos in range(4):
                    r = 3 - pos
                    nc.sync.dma_start(
                        v_view[:, 2 * gi:2 * gi + 2, r, :],
                        ago[pos, 2 * SLOT_K:2 * SLOT_K + 2 * SLOT_V]
                        .rearrange("(si p w) -> p si w", si=2, p=P)
                        .bitcast(BF16))
                    for si in range(2):
                        j = (2 * gi + si) * 4 + r
                        nc.sync.dma_start(
                            kT[:, :, j * P:(j + 1) * P],
                            ago[pos, si * SLOT_K:(si + 1) * SLOT_K]
                            .rearrange("(d g w) -> d g w", d=P, g=KVH)
                            .bitcast(BF16))

            wo_sb = wopool.tile([P, HB, HID], BF16, name="wo_sb")
            for c in range(4):
                HBQ = HB // 4
                WO4 = HBQ * HID
                nc.sync.dma_start(
                    wo_sb[:, c * HBQ:(c + 1) * HBQ, :],
                    wo_p[:, c * WO4:(c + 1) * WO4]
                    .rearrange("p (hb n) -> p hb n", hb=HBQ))

            with tc.tile_pool(name="ppool", bufs=3) as ppool, \
                 tc.tile_pool(name="astage", bufs=2) as astage, \
                 tc.tile_pool(name="ctxp", bufs=1) as ctxp, \
                 tc.tile_pool(name="ostage", bufs=3) as ostage, \
                 tc.tile_pool(name="scps", bufs=3, space="PSUM") as scps, \
                 tc.tile_pool(name="cps", bufs=2, space="PSUM") as cps, \
                 tc.tile_pool(name="rps", bufs=1, space="PSUM") as rps, \
                 tc.tile_pool(name="ops", bufs=2, space="PSUM") as ops:
                OC = HID // 4

                def wo_chunk(ws, oc, wctx):
                    po = ops.tile([P, OC], FP32, name="po", tag="po")
                    for wg in range(KVH):
                        for wh in range(GPQ):
                            hh = wg * GPQ + wh
                            nc.tensor.matmul(po[:], wctx[:, wg, wh, :],
                                             wo_sb[:, hh, oc * OC:(oc + 1) * OC],
                                             start=(hh == 0), stop=(hh == HB - 1))
                    ot = ostage.tile([P, OC], FP32, name="ot", tag="ot")
                    nc.vector.tensor_copy(ot[:], po[:])
                    nc.sync.dma_start(
                        out_p[ws * P:(ws + 1) * P, oc * OC:(oc + 1) * OC], ot[:])

                for s in range(NSLOT):
                    Es = E[s]
                    ctx_s = ctxp.tile([P, KVH, GPQ, P], BF16, name=f"ctx{s}",
                                      tag=f"ctx{s % 2}")
                    for g in range(KVH):
                        q_rhs = qT[:, g * GPQ:(g + 1) * GPQ, s * P:(s + 1) * P]
                        pctx = cps.tile([P, GPQ * P], FP32, name="pctx", tag="pctx")
                        prs = rps.tile([P, GPQ * P], FP32, name="prs", tag="prs")
                        pts = []
                        for kb in range(Es):
                            psc = scps.tile([P, GPQ * P], FP32, name="psc", tag="psc")
                            mi = mask_idx.get((s, kb))
                            nc.tensor.matmul(
                                psc[:], kT[:, g, kb * P:(kb + 1) * P],
                                q_rhs, start=True, stop=True)
                            if mi is not None:
                                psc3 = psc[:].rearrange("p (h t) -> p h t", t=P)
                                nc.vector.tensor_tensor(
                                    psc3, psc3,
                                    masks_t[:, mi, None, :]
                                    .to_broadcast((P, GPQ, P)),
                                    mybir.AluOpType.add)
                            pt = ppool.tile([P, GPQ * P], BF16, name="pt", tag="pt")
                            nc.scalar.activation(pt[:], psc[:], Exp)
                            nc.tensor.matmul(pctx[:],
                                             v_all[:, kb, g * HD:(g + 1) * HD],
                                             pt[:],
                                             start=(kb == 0), stop=(kb == Es - 1))
                            pts.append(pt)
                            if kb % 4 == 3:
                                pa = ppool.tile([P, GPQ * P], BF16,
                                                name="pa", tag="pa")
                                pb = ppool.tile([P, GPQ * P], BF16,
                                                name="pb", tag="pb")
                                nc.vector.tensor_tensor(pa[:], pts[-4][:],
                                                        pts[-3][:],
                                                        mybir.AluOpType.add)
                                nc.vector.tensor_tensor(pb[:], pa[:], pts[-2][:],
                                                        mybir.AluOpType.add)
                                nc.vector.tensor_tensor(pa[:], pb[:], pts[-1][:],
                                                        mybir.AluOpType.add)
                                nc.tensor.matmul(prs[:], ones_bf[:], pa[:],
                                                 start=(kb == 3),
                                                 stop=(kb == Es - 1))
                                pts = []
                        for i, pt in enumerate(pts):
                            nc.tensor.matmul(prs[:], ones_bf[:], pt[:],
                                             start=(Es < 4 and i == 0),
                                             stop=(i == len(pts) - 1))
                        rr = astage.tile([P, GPQ * P], FP32, name="rr", tag="rr")
                        nc.vector.reciprocal_approx_fast(rr[:], prs[:])
                        nc.vector.tensor_tensor(
                            ctx_s[:, g, :, :],
                            pctx[:].rearrange("p (h t) -> p h t", t=P),
                            rr[:].rearrange("p (h t) -> p h t", t=P),
                            mybir.AluOpType.mult)
                        # fill exp-wait gaps with the previous slot's Wo chunk
                        if s > 0:
                            wo_chunk(s - 1, g, ctx_prev)
                    ctx_prev = ctx_s
                for oc in range(4):
                    wo_chunk(NSLOT - 1, oc, ctx_prev)

    nc.compile()
    return nc


def _prep_inputs(hidden_states, attention_mask, cos, sin, Wq, Wk, Wv, Wo, P_list):
    hs = np.ascontiguousarray(np.asarray(hidden_states, dtype=np.float32))
    mask = np.asarray(attention_mask, dtype=np.float32).reshape(T, T)
    cos2 = np.asarray(cos, dtype=np.float32).reshape(T, HD)
    sin2 = np.asarray(sin, dtype=np.float32).reshape(T, HD)
    scale = np.float32(1.0 / np.sqrt(HD))

    def t3(s_):
        # rotate_half add trick: t3 = concat(sin[:, 64:], -sin[:, :64])
        return np.concatenate([s_[:, HD // 2:], -s_[:, :HD // 2]], axis=1)

    bf = ml_dtypes.bfloat16

    def wprep(w):
        w = np.asarray(w, dtype=np.float32).astype(bf)
        n = w.shape[1]
        return np.ascontiguousarray(
            w.reshape(HB, P, n).transpose(1, 0, 2).reshape(P, HB * n))

    wq = wprep(Wq)
    wk = wprep(Wk)
    wv = wprep(Wv)
    wo = wprep(Wo)

    in_maps = []
    for i in range(NC):
        b, pos = i // 4, i % 4
        js = [4 * s + 3 - pos for s in range(NSLOT)]
        take = lambda a: np.ascontiguousarray(
            np.concatenate([a[j * P:(j + 1) * P] for j in js], axis=0))
        m_tiles = [mask[js[s] * P:(js[s] + 1) * P, kb * P:(kb + 1) * P].T
                   for (s, kb) in P_list]
        if not m_tiles:
            m_tiles.append(np.zeros((P, P), np.float32))
        xc = take(hs[b])                         # [512, 2048] fp32
        xt = np.ascontiguousarray(
            xc.T.reshape(HB, P, NSLOT * P).transpose(1, 0, 2)
            .reshape(P, HB * NSLOT * P).astype(bf))
        in_maps.append({
            "x": xt,
            "wq": wq, "wk": wk, "wv": wv, "wo": wo,
            "cosq": take(cos2 * scale),
            "sinq3": take(t3(sin2 * scale)),
            "cosk": take(cos2),
            "sink3": take(t3(sin2)),
            "masks": np.stack(m_tiles).astype(bf),
        })
    return in_maps


_cache = {}


def kernel(hidden_states, attention_mask, cos, sin, Wq, Wk, Wv, Wo,
           _trace=False, _trace_kwargs=None):
    from concourse.bass_utils import run_bass_kernel_spmd

    E, P_list = _mask_plan(attention_mask)
    key = (tuple(E), tuple(P_list))
    if key not in _cache:
        _cache[key] = _build_program(E, P_list)
    nc = _cache[key]

    in_maps = _prep_inputs(hidden_states, attention_mask, cos, sin,
                           Wq, Wk, Wv, Wo, P_list)
    kwargs = dict(_trace_kwargs or {})
    if _trace:
        kwargs["trace"] = True
    res = run_bass_kernel_spmd(nc, in_maps, list(range(NC)), **kwargs)

    out = np.empty((B, T, HID), dtype=np.float32)
    for i in range(NC):
        b, pos = i // 4, i % 4
        o = res.results[i]["out"]
        for s in range(NSLOT):
            j = 4 * s + 3 - pos
            out[b, j * P:(j + 1) * P, :] = o[s * P:(s + 1) * P, :]
    kernel._last_result = res
    return out


# revision 23
# speedup vs baseline: 1.1516x; 1.0337x over previous
# Trainium2 Bass kernel for AvaAttention (GQA attention + RoPE + additive mask)
# B=2, T=2048, HID=2048, NH=16, KVH=4, HD=128, fp32 — 8 NeuronCores.
#
# Sharding: sequence-parallel. Core i (batch b=i//4, position p=i%4) owns
# q-blocks j = 4s+3-p of batch b, for slot s in 0..3. Projections are
# row-parallel (weights replicated, bf16), K/V exchanged with a SINGLE
# combined AllGather (bf16 payloads packed in a flat fp32 buffer) over
# each batch's 4 cores; attention + output projection stay local.
#
# v3 notes:
#  - Projections run in bf16 (bf16 xT + bf16 weights); RoPE in fp32 from
#    the fp32 PSUM projection result; q/k re-cast to bf16 on the
#    PSUM->SBUF eviction after their transposes.
#  - Scores are computed pre-transposed ([tk, (h tq)]): K block is the
#    stationary operand, 4 q-heads stream at once (N=512). No per-head
#    diag/transpose matmuls, no PSUM->bf16 CAST of probabilities.
#  - Softmax denominators via an all-ones stationary matmul accumulated
#    over kb; result is replicated across partitions so normalization is
#    one elementwise multiply fused into the ctx PSUM->SBUF eviction.
#    1/x via the fast custom-DVE reciprocal (plain reciprocal is ~3.4us
#    per tile and serialized the attention tail).
#  - Additive mask: one N=512 matmul per masked tile (lhsT = mask data in
#    natural [tq, tk], rhs = 4 identity blocks).
#  - Wo is bf16, fully resident in SBUF, and its matmuls interleave with
#    attention per-slot so the tensor engine stays busy to the end.
#  - exp without max-subtraction (safe at this score scale; masked
#    positions hit exp(S-1e9)=0).

import sys

for _p in ("/opt/trn_rl_repo", "/opt/pypackages"):
    if _p not in sys.path:
        sys.path.insert(0, _p)

import numpy as np
import ml_dtypes

B, T, HID = 2, 2048, 2048
NH, KVH, HD = 16, 4, 128
P = 128
NC = 8
NBLK = T // P          # 16 q-blocks per batch
NSLOT = 4              # blocks per core
GPQ = NH // KVH        # 4 q-heads per kv group
HB = HID // P          # 16 contraction subtiles
NEG_THRESH = -1.0e8
KVW = KVH * HD         # 512
SLOT_K = P * KVH * P // 2        # 32768 fp32 words: one slot's bf16 kT
SLOT_V = P * KVW // 2            # 32768 fp32 words: one slot's bf16 V
AG_K = NSLOT * SLOT_K            # 131072
AG_V = NSLOT * SLOT_V            # 131072


def _mask_plan(attention_mask):
    """Classify the additive mask per (j, kb) 128x128 tile.

    Returns (E, P_list): E[s] is the uniform k-extent (in blocks) for slot
    s; P_list is the ordered list of (s, kb) positions where a mask-add is
    applied (positions shared by every core; tile *data* is per-core).
    """
    m = np.asarray(attention_mask).reshape(T, T)
    nonzero = np.zeros((NBLK, NBLK), dtype=bool)
    live = np.zeros((NBLK, NBLK), dtype=bool)   # not fully masked
    for j in range(NBLK):
        for kb in range(NBLK):
            tile = m[j * P:(j + 1) * P, kb * P:(kb + 1) * P]
            nonzero[j, kb] = bool(np.any(tile != 0.0))
            live[j, kb] = bool(np.any(tile > NEG_THRESH))
    kmax = np.ones(NBLK, dtype=int)
    for j in range(NBLK):
        idx = np.nonzero(live[j])[0]
        if len(idx):
            kmax[j] = int(idx[-1]) + 1
    E = [int(max(kmax[4 * s + jj] for jj in range(4))) for s in range(NSLOT)]
    P_list = []
    for s in range(NSLOT):
        for kb in range(E[s]):
            if any(nonzero[4 * s + jj, kb] for jj in range(4)):
                P_list.append((s, kb))
    return E, P_list


def _build_program(E, P_list):
    import concourse.mybir as mybir
    import concourse.tile as tile
    from concourse import bacc
    from concourse.masks import make_identity
    from contextlib import ExitStack

    FP32 = mybir.dt.float32
    FP32R = mybir.dt.float32r
    BF16 = mybir.dt.bfloat16
    FP8 = mybir.dt.float8e4
    DR = mybir.MatmulPerfMode.DoubleRow
    Exp = mybir.ActivationFunctionType.Exp
    HALF = HD // 2

    nc = bacc.Bacc("TRN2", target_bir_lowering=False, num_devices=NC)

    x_p = nc.declare_dram_parameter("x", [P, HB * NSLOT * P], BF16, isOutput=False)
    wq_p = nc.declare_dram_parameter("wq", [P, HB * NH * HD], BF16, isOutput=False)
    wk_p = nc.declare_dram_parameter("wk", [P, HB * KVH * HD], BF16, isOutput=False)
    wv_p = nc.declare_dram_parameter("wv", [P, HB * KVH * HD], BF16, isOutput=False)
    wo_p = nc.declare_dram_parameter("wo", [P, HB * HID], BF16, isOutput=False)
    cosq_p = nc.declare_dram_parameter("cosq", [NSLOT * P, HD], FP32, isOutput=False)
    sinq_p = nc.declare_dram_parameter("sinq3", [NSLOT * P, HD], FP32, isOutput=False)
    cosk_p = nc.declare_dram_parameter("cosk", [NSLOT * P, HD], FP32, isOutput=False)
    sink_p = nc.declare_dram_parameter("sink3", [NSLOT * P, HD], FP32, isOutput=False)
    nmask = max(1, len(P_list))
    masks_p = nc.declare_dram_parameter("masks", [nmask, P, P], BF16, isOutput=False)
    out_p = nc.declare_dram_parameter("out", [NSLOT * P, HID], FP32, isOutput=True)

    HALF_AG = 2 * (SLOT_K + SLOT_V)
    ag_in1 = nc.dram_tensor("ag_in1", [HALF_AG], FP32)
    ag_out1 = nc.dram_tensor("ag_out1", [4, HALF_AG], FP32, addr_space="Local")
    ag_in2 = nc.dram_tensor("ag_in2", [HALF_AG], FP32)
    ag_out2 = nc.dram_tensor("ag_out2", [4, HALF_AG], FP32, addr_space="Local")
    groups = [[0, 1, 2, 3], [4, 5, 6, 7]]

    mask_idx = {sk: idx for idx, sk in enumerate(P_list)}

    def rope(engine, dst, src_ps, cos_t, sin_t, s, nh):
        """dst[t, h, d] = src*cos + rotate_half(src)*sin, natural layout."""
        src3 = src_ps[:].rearrange("p (h d) -> p h d", d=HD)
        cst = rope.pool.tile([P, nh, HD], FP32, name="rope_c", tag="rope_c")
        engine.tensor_tensor(dst[:], src3,
                             cos_t[:, s, None, :].to_broadcast((P, nh, HD)),
                             mybir.AluOpType.mult)
        engine.tensor_tensor(cst[:], src3,
                             sin_t[:, s, None, :].to_broadcast((P, nh, HD)),
                             mybir.AluOpType.mult)
        engine.tensor_tensor(dst[:, :, HALF:], dst[:, :, HALF:],
                             cst[:, :, :HALF], mybir.AluOpType.add)
        engine.tensor_tensor(dst[:, :, :HALF], dst[:, :, :HALF],
                             cst[:, :, HALF:], mybir.AluOpType.add)

    with tile.TileContext(nc) as tc, ExitStack() as top:
        const = top.enter_context(tc.tile_pool(name="const", bufs=1))
        ident_f32 = const.tile([P, P], FP32)
        make_identity(nc, ident_f32[:])
        ones_bf = const.tile([P, P], BF16)
        nc.gpsimd.memset(ones_bf[:], 1.0)

        cosq_t = const.tile([P, NSLOT, HD], FP32)
        sinq_t = const.tile([P, NSLOT, HD], FP32)
        cosk_t = const.tile([P, NSLOT, HD], FP32)
        sink_t = const.tile([P, NSLOT, HD], FP32)
        masks_t = const.tile([P, nmask, P], BF16)

        qT_pool = top.enter_context(tc.tile_pool(name="qT_pool", bufs=1))
        qT = qT_pool.tile([P, NH, NSLOT * P], BF16)           # [d, h, t]

        # ================= projection phases =================
        with tc.tile_pool(name="xT_pool", bufs=1) as xT_pool, \
             tc.tile_pool(name="qw", bufs=1) as qw_pool, \
             tc.tile_pool(name="ph0ps", bufs=2, space="PSUM") as ps0:
            xT = xT_pool.tile([P, HB, NSLOT * P], BF16)       # [h%128, hb, t]
            wq_sb = qw_pool.tile([P, HB, NH * HD], BF16, name="wq_sb")
            # x arrives pre-transposed (host-side) as [p, hb, t] bf16
            for c in range(4):
                HBH = HB // 4
                WH = HBH * NSLOT * P
                nc.sync.dma_start(
                    xT[:, c * HBH:(c + 1) * HBH, :],
                    x_p[:, c * WH:(c + 1) * WH]
                    .rearrange("p (hb t) -> p hb t", hb=HBH))

            # ---- phase 1a: K/V proj + RoPE + per-slot staging + AllGather ----
            with tc.tile_pool(name="kvw", bufs=1) as kvw_pool, \
                 tc.tile_pool(name="kvstage", bufs=2) as kvstage, \
                 tc.tile_pool(name="ktps", bufs=2, space="PSUM") as ktps:
                rope.pool = kvstage
                wk_sb = kvw_pool.tile([P, HB, KVW], BF16, name="wk_sb")
                wv_sb = kvw_pool.tile([P, HB, KVW], BF16, name="wv_sb")
                for c in range(2):
                    HBH = HB // 2
                    WH = HBH * KVW
                    nc.sync.dma_start(
                        wk_sb[:, c * HBH:(c + 1) * HBH, :],
                        wk_p[:, c * WH:(c + 1) * WH]
                        .rearrange("p (hb n) -> p hb n", hb=HBH))
                    nc.sync.dma_start(
                        wv_sb[:, c * HBH:(c + 1) * HBH, :],
                        wv_p[:, c * WH:(c + 1) * WH]
                        .rearrange("p (hb n) -> p hb n", hb=HBH))
                for ap, prm in ((cosk_t, cosk_p), (sink_t, sink_p),
                                (cosq_t, cosq_p), (sinq_t, sinq_p)):
                    nc.sync.dma_start(ap[:], prm[:].rearrange("(s p) d -> p s d", p=P))
                nc.sync.dma_start(masks_t[:], masks_p[:].rearrange("n p d -> p n d"))
                for c in range(8):
                    HBO = HB // 8
                    WQ8 = HBO * NH * HD
                    nc.sync.dma_start(
                        wq_sb[:, c * HBO:(c + 1) * HBO, :],
                        wq_p[:, c * WQ8:(c + 1) * WQ8]
                        .rearrange("p (hb n) -> p hb n", hb=HBO))

                for s in range(NSLOT):
                    pk = ps0.tile([P, KVW], FP32, name="pk", tag="pkv")
                    for hb in range(HB):
                        nc.tensor.matmul(pk[:], xT[:, hb, s * P:(s + 1) * P],
                                         wk_sb[:, hb, :],
                                         start=(hb == 0), stop=(hb == HB - 1))
                    kr = kvstage.tile([P, KVH, HD], FP32, name=f"k_rope{s}",
                                      tag=f"k_rope{s % 2}")
                    rope(nc.vector, kr, pk, cosk_t, sink_t, s, KVH)

                    pv = ps0.tile([P, KVW], FP32, name="pv", tag="pkv")
                    for hb in range(HB):
                        nc.tensor.matmul(pv[:], xT[:, hb, s * P:(s + 1) * P],
                                         wv_sb[:, hb, :],
                                         start=(hb == 0), stop=(hb == HB - 1))
                    vst = kvstage.tile([P, KVW], BF16, name=f"v_st{s}", tag="v_st")
                    nc.vector.tensor_copy(vst[:], pv[:])
                    agi, si = (ag_in1, s) if s < 2 else (ag_in2, s - 2)
                    nc.sync.dma_start(
                        agi[2 * SLOT_K + si * SLOT_V:
                            2 * SLOT_K + (si + 1) * SLOT_V]
                        .rearrange("(p w) -> p w", p=P),
                        vst[:].bitcast(FP32))

                    # transpose this slot's k and stage it (bf16)
                    pkt = ktps.tile([P, KVH * P], FP32, name="pkt", tag="pkt")
                    for g in range(KVH):
                        nc.tensor.transpose(pkt[:, g * P:(g + 1) * P],
                                            kr[:, g, :], ident_f32[:])
                    kst = kvstage.tile([P, KVH, P], BF16, name=f"k_st{s}",
                                       tag="k_st")
                    nc.vector.tensor_copy(
                        kst[:], pkt[:].rearrange("p (g t) -> p g t", t=P))
                    nc.sync.dma_start(
                        agi[si * SLOT_K:(si + 1) * SLOT_K]
                        .rearrange("(d g w) -> d g w", d=P, g=KVH),
                        kst[:].bitcast(FP32))
                    if s == 1:
                        nc.gpsimd.collective_compute(
                            "AllGather", mybir.AluOpType.bypass,
                            replica_groups=groups,
                            ins=[ag_in1[:]], outs=[ag_out1[:]])
                    elif s == 3:
                        nc.gpsimd.collective_compute(
                            "AllGather", mybir.AluOpType.bypass,
                            replica_groups=groups,
                            ins=[ag_in2[:]], outs=[ag_out2[:]])

            # ---- phase 1b: Q projection + RoPE + transpose to qT ----
            QC = 4  # heads per Wq chunk
            with tc.tile_pool(name="qstage", bufs=3) as qstage, \
                 tc.tile_pool(name="qps", bufs=2, space="PSUM") as qps, \
                 tc.tile_pool(name="qtps", bufs=2, space="PSUM") as qtps:
                rope.pool = qstage
                for hc in range(NH // QC):
                    q_rope = []
                    for s in range(NSLOT):
                        pq = qps.tile([P, QC * HD], FP32, name="pq", tag="pq")
                        for hb in range(HB):
                            nc.tensor.matmul(pq[:], xT[:, hb, s * P:(s + 1) * P],
                                             wq_sb[:, hb,
                                                   hc * QC * HD:(hc + 1) * QC * HD],
                                             start=(hb == 0), stop=(hb == HB - 1))
                        qr = qstage.tile([P, QC, HD], FP32, name=f"q_rope{s}",
                                         tag=f"q_rope{s % 2}")
                        rope(nc.vector, qr, pq, cosq_t, sinq_t, s, QC)
                        q_rope.append(qr)
                    for h in range(QC):
                        pqt = qtps.tile([P, NSLOT * P], FP32, name="pqt", tag="pqt")
                        for s in range(NSLOT):
                            nc.tensor.transpose(pqt[:, s * P:(s + 1) * P],
                                                q_rope[s][:, h, :], ident_f32[:])
                        nc.vector.tensor_copy(qT[:, hc * QC + h, :], pqt[:])

        # ================= gather + attention + interleaved Wo =================
        with tc.tile_pool(name="kv_pool", bufs=1) as kv_pool, \
             tc.tile_pool(name="wopool", bufs=1) as wopool:
            kT1 = kv_pool.tile([P, KVH, T // 2], BF16)    # [d, g, t] blocks 0-7
            kT2 = kv_pool.tile([P, KVH, T // 2], BF16)    # blocks 8-15
            v_all1 = kv_pool.tile([P, NBLK // 2, KVW], BF16)
            v_all2 = kv_pool.tile([P, NBLK // 2, KVW], BF16)

            # block j = 4s + (3-pos); gather both slots of a group per DMA
            wo_sb = wopool.tile([P, HB, HID], BF16, name="wo_sb")
            for gi, ago in ((0, ag_out1), (1, ag_out2)):
                vt, kt = (v_all1, kT1) if gi == 0 else (v_all2, kT2)
                v_view = vt[:].rearrange("p (s r) w -> p s r w", r=4)
                for pos in range(4):
                    r = 3 - pos
                    nc.sync.dma_start(
                        v_view[:, :, r, :],
                        ago[pos, 2 * SLOT_K:2 * SLOT_K + 2 * SLOT_V]
                        .rearrange("(si p w) -> p si w", si=2, p=P)
                        .bitcast(BF16))
                    for si in range(2):
                        jl = si * 4 + r
                        nc.sync.dma_start(
                            kt[:, :, jl * P:(jl + 1) * P],
                            ago[pos, si * SLOT_K:(si + 1) * SLOT_K]
                            .rearrange("(d g w) -> d g w", d=P, g=KVH)
                            .bitcast(BF16))
                if gi == 0:
                    for c in range(4):
                        HBQ = HB // 4
                        WO4 = HBQ * HID
                        nc.sync.dma_start(
                            wo_sb[:, c * HBQ:(c + 1) * HBQ, :],
                            wo_p[:, c * WO4:(c + 1) * WO4]
                            .rearrange("p (hb n) -> p hb n", hb=HBQ))

            with tc.tile_pool(name="ppool", bufs=3) as ppool, \
                 tc.tile_pool(name="astage", bufs=2) as astage, \
                 tc.tile_pool(name="ctxp", bufs=1) as ctxp, \
                 tc.tile_pool(name="ostage", bufs=3) as ostage, \
                 tc.tile_pool(name="scps", bufs=3, space="PSUM") as scps, \
                 tc.tile_pool(name="cps", bufs=2, space="PSUM") as cps, \
                 tc.tile_pool(name="rps", bufs=1, space="PSUM") as rps, \
                 tc.tile_pool(name="ops", bufs=2, space="PSUM") as ops:
                OC = HID // 4

                def wo_chunk(ws, oc, wctx):
                    po = ops.tile([P, OC], FP32, name="po", tag="po")
                    for wg in range(KVH):
                        for wh in range(GPQ):
                            hh = wg * GPQ + wh
                            nc.tensor.matmul(po[:], wctx[:, wg, wh, :],
                                             wo_sb[:, hh, oc * OC:(oc + 1) * OC],
                                             start=(hh == 0), stop=(hh == HB - 1))
                    ot = ostage.tile([P, OC], FP32, name="ot", tag="ot")
                    nc.vector.tensor_copy(ot[:], po[:])
                    nc.sync.dma_start(
                        out_p[ws * P:(ws + 1) * P, oc * OC:(oc + 1) * OC], ot[:])

                for s in range(NSLOT):
                    Es = E[s]
                    ctx_s = ctxp.tile([P, KVH, GPQ, P], BF16, name=f"ctx{s}",
                                      tag=f"ctx{s % 2}")
                    for g in range(KVH):
                        q_rhs = qT[:, g * GPQ:(g + 1) * GPQ, s * P:(s + 1) * P]
                        pctx = cps.tile([P, GPQ * P], FP32, name="pctx", tag="pctx")
                        prs = rps.tile([P, GPQ * P], FP32, name="prs", tag="prs")
                        pts = []
                        for kb in range(Es):
                            psc = scps.tile([P, GPQ * P], FP32, name="psc", tag="psc")
                            mi = mask_idx.get((s, kb))
                            kt_g = kT1 if kb < 8 else kT2
                            kbl = kb % 8
                            nc.tensor.matmul(
                                psc[:], kt_g[:, g, kbl * P:(kbl + 1) * P],
                                q_rhs, start=True, stop=True)
                            if mi is not None:
                                psc3 = psc[:].rearrange("p (h t) -> p h t", t=P)
                                nc.vector.tensor_tensor(
                                    psc3, psc3,
                                    masks_t[:, mi, None, :]
                                    .to_broadcast((P, GPQ, P)),
                                    mybir.AluOpType.add)
                            pt = ppool.tile([P, GPQ * P], BF16, name="pt", tag="pt")
                            nc.scalar.activation(pt[:], psc[:], Exp)
                            vt_g = v_all1 if kb < 8 else v_all2
                            nc.tensor.matmul(pctx[:],
                                             vt_g[:, kbl, g * HD:(g + 1) * HD],
                                             pt[:],
                                             start=(kb == 0), stop=(kb == Es - 1))
                            pts.append(pt)
                            if kb % 4 == 3:
                                pa = ppool.tile([P, GPQ * P], BF16,
                                                name="pa", tag="pa")
                                pb = ppool.tile([P, GPQ * P], BF16,
                                                name="pb", tag="pb")
                                nc.vector.tensor_tensor(pa[:], pts[-4][:],
                                                        pts[-3][:],
                                                        mybir.AluOpType.add)
                                nc.vector.tensor_tensor(pb[:], pa[:], pts[-2][:],
                                                        mybir.AluOpType.add)
                                nc.vector.tensor_tensor(pa[:], pb[:], pts[-1][:],
                                                        mybir.AluOpType.add)
                                nc.tensor.matmul(prs[:], ones_bf[:], pa[:],
                                                 start=(kb == 3),
                                                 stop=(kb == Es - 1))
                                pts = []
                        for i, pt in enumerate(pts):
                            nc.tensor.matmul(prs[:], ones_bf[:], pt[:],
                                             start=(Es < 4 and i == 0),
                                             stop=(i == len(pts) - 1))
                        rr = astage.tile([P, GPQ * P], FP32, name="rr", tag="rr")
                        nc.vector.reciprocal_approx_fast(rr[:], prs[:])
                        nc.vector.tensor_tensor(
                            ctx_s[:, g, :, :],
                            pctx[:].rearrange("p (h t) -> p h t", t=P),
                            rr[:].rearrange("p (h t) -> p h t", t=P),
                            mybir.AluOpType.mult)
                        # fill exp-wait gaps with the previous slot's Wo chunk
                        if s > 0:
                            wo_chunk(s - 1, g, ctx_prev)
                    ctx_prev = ctx_s
                for oc in range(4):
                    wo_chunk(NSLOT - 1, oc, ctx_prev)

    nc.compile()
    return nc


def _prep_inputs(hidden_states, attention_mask, cos, sin, Wq, Wk, Wv, Wo, P_list):
    hs = np.ascontiguousarray(np.asarray(hidden_states, dtype=np.float32))
    mask = np.asarray(attention_mask, dtype=np.float32).reshape(T, T)
    cos2 = np.asarray(cos, dtype=np.float32).reshape(T, HD)
    sin2 = np.asarray(sin, dtype=np.float32).reshape(T, HD)
    scale = np.float32(1.0 / np.sqrt(HD))

    def t3(s_):
        # rotate_half add trick: t3 = concat(sin[:, 64:], -sin[:, :64])
        return np.concatenate([s_[:, HD // 2:], -s_[:, :HD // 2]], axis=1)

    bf = ml_dtypes.bfloat16

    def wprep(w):
        w = np.asarray(w, dtype=np.float32).astype(bf)
        n = w.shape[1]
        return np.ascontiguousarray(
            w.reshape(HB, P, n).transpose(1, 0, 2).reshape(P, HB * n))

    wq = wprep(Wq)
    wk = wprep(Wk)
    wv = wprep(Wv)
    wo = wprep(Wo)

    in_maps = []
    for i in range(NC):
        b, pos = i // 4, i % 4
        js = [4 * s + 3 - pos for s in range(NSLOT)]
        take = lambda a: np.ascontiguousarray(
            np.concatenate([a[j * P:(j + 1) * P] for j in js], axis=0))
        m_tiles = [mask[js[s] * P:(js[s] + 1) * P, kb * P:(kb + 1) * P].T
                   for (s, kb) in P_list]
        if not m_tiles:
            m_tiles.append(np.zeros((P, P), np.float32))
        xc = take(hs[b])                         # [512, 2048] fp32
        xt = np.ascontiguousarray(
            xc.T.reshape(HB, P, NSLOT * P).transpose(1, 0, 2)
            .reshape(P, HB * NSLOT * P).astype(bf))
        in_maps.append({
            "x": xt,
            "wq": wq, "wk": wk, "wv": wv, "wo": wo,
            "cosq": take(cos2 * scale),
            "sinq3": take(t3(sin2 * scale)),
            "cosk": take(cos2),
            "sink3": take(t3(sin2)),
            "masks": np.stack(m_tiles).astype(bf),
        })
    return in_maps


_cache = {}


def kernel(hidden_states, attention_mask, cos, sin, Wq, Wk, Wv, Wo,
           _trace=False, _trace_kwargs=None):
    from concourse.bass_utils import run_bass_kernel_spmd

    E, P_list = _mask_plan(attention_mask)
    key = (tuple(E), tuple(P_list))
    if key not in _cache:
        _cache[key] = _build_program(E, P_list)
    nc = _cache[key]

    in_maps = _prep_inputs(hidden_states, attention_mask, cos, sin,
                           Wq, Wk, Wv, Wo, P_list)
    kwargs = dict(_trace_kwargs or {})
    if _trace:
        kwargs["trace"] = True
    res = run_bass_kernel_spmd(nc, in_maps, list(range(NC)), **kwargs)

    out = np.empty((B, T, HID), dtype=np.float32)
    for i in range(NC):
        b, pos = i // 4, i % 4
        o = res.results[i]["out"]
        for s in range(NSLOT):
            j = 4 * s + 3 - pos
            out[b, j * P:(j + 1) * P, :] = o[s * P:(s + 1) * P, :]
    kernel._last_result = res
    return out
